# revision 15
# baseline (speedup 1.0000x reference)
"""Trainium2 Bass kernel for nn_Attention_54391465836966.

Math (per batch b, component n; one (b,n) pair per core, host sums over n):
  ctok = content_feat[b].raw_reshape(S,C) + pos          # [1024, 512]
  comp_tok = components[n,b].raw_reshape(S,C) + pos
  q = ctok @ Wq ; k,v = comp_tok @ Wkv (split)
  per head h: P = exp(scale q_h k_h^T); o_h = (P @ v_h) / rowsum(P)
  s_in = concat_h(o_h) + gate*ctok                        # gate = [n==0]
  s2d  = raw_reshape(s_in @ Wproj)                        # bias via host
  out  = Wconv^T[:C] @ s2d + Wconv^T[C:] @ cf             # + bias via host

Design notes (cost-model-driven):
- The P@V (o) matmuls run fp8 DoubleRow (2 k-tiles per instruction at 0.5
  cycles/row = 4x the bf16 rate): pt = exp output in e5m2 (its 22-efold
  dynamic range covers the unnormalized exp spread -- score sigma ~2.0 ->
  +-11 efolds; e4m3's 11.7 efolds would overflow), v drained to e4m3.
  Attention-output noise from fp8 is diluted by the ctok residual and
  P-quantization partially cancels against Z (measured 1.4e-2 total).
- The q/k path stays bf16 end-to-end: fp8 anywhere before the exp adds
  ~0.17 absolute score noise -> 17% P-reweighting with only ~19 effective
  keys after softmax (score sigma 2.0) -> 3-4e-2 output error (measured).
- exp writes pt e5m2 directly with a constant -4 bias (normalization
  cancels it exactly; caps pt at exp(score-4) with overflow only past
  7.5 sigma scores, flush below -7.1 negligible).
- Z rides row 0 of the o PSUM tile (ones column first in v): the custom-DVE
  reciprocal reads partition 0 of PSUM directly (hw quirk: it ignores the
  AP's partition base), skipping the per-head z-copy entirely.  Drain +
  normalize fuse into one scalar_tensor_tensor from PSUM (v data sits at
  e=64:128 so the o data rows land at the 32-aligned partition base 64).
- The projection uses stride-2 column slices of s_in^T as the stationary
  operand, which makes its PSUM output land directly in the s->s2d
  raw-reshape layout: no PE transposes anywhere in the kernel.
- bproj/bconv are affine constants independent of the data path; their
  contribution is bconv[o] + (sum_c Wconv[o,c<C]) * bproj[i%512], a rank-1
  image the host adds during unshard.
- PSUM: score tiles tag "sc" [128,1024] bufs=3 (6 banks) shared by the
  QKV-projection accumulations (pre-phase + interleaved jobs borrow
  rotation slots); o tag [128,1024] bufs=1 (2 banks).  GPSIMD cannot touch
  PSUM, so all PSUM drains go to DVE (ACT helps only post-exp in the tail).
- ACT runs exps only (64 x [128,1024] ~= 1.04us each); QKV/conv_cf jobs are
  split into ~2048-cycle half-jobs and interleaved between score matmuls so
  the exp stream never starves.  o matmuls for head h trail into head h+1's
  first slots; the recip->broadcast->normalize chain then frees the single
  o buffer before head h+1 needs it.
"""
import sys

sys.path.insert(0, "/opt/trn_rl_repo")

import numpy as np

N_CORES = 8
B, C, H, W = 2, 512, 32, 32
S = H * W  # 1024
NH, HD = 8, 64
SCALE = HD ** -0.5
EXP_BIAS = -4.0

_CACHE = {}


def _build():
    if "nc" in _CACHE:
        return _CACHE["nc"]
    from contextlib import ExitStack

    import concourse.bacc as bacc
    import concourse.mybir as mybir
    import concourse.tile as tile

    f32 = mybir.dt.float32
    bf16 = mybir.dt.bfloat16
    fp8 = mybir.dt.float8e4
    fp8e5 = mybir.dt.float8e5
    EXP = mybir.ActivationFunctionType.Exp
    MULT = mybir.AluOpType.mult
    ADD = mybir.AluOpType.add
    BYPASS = mybir.AluOpType.bypass
    DR = mybir.MatmulPerfMode.DoubleRow

    nc = bacc.Bacc("TRN2", target_bir_lowering=False, debug=False,
                   num_devices=N_CORES)

    din = lambda n, s, dt: nc.dram_tensor(n, s, dt, kind="ExternalInput").ap()
    comp16d = din("comp16", [C, S], bf16)   # (comp_tok + pos).T
    ctok16d = din("ctok16", [C, S], bf16)   # (content_tok + pos).T
    cf16d = din("cf16", [C, S], bf16)       # content_feat[b] raw [C,S]
    wq16d = din("wq16", [C, C], bf16)       # Wq
    wkv16d = din("wkv16", [C, 2 * C], bf16)  # Wkv (cols 0:C K, C:2C V)
    wp16d = din("wp16", [C, C], bf16)       # Wproj
    wcs16d = din("wcs16", [C, C], bf16)     # Wconv.T rows :C (s half)
    wcc16d = din("wcc16", [C, 128], bf16)   # Wconv.T[C:, 128n:128(n+1)]
    gated = din("gate", [128, 1], f32)      # 1.0 on n==0 cores else 0.0
    out_p = nc.dram_tensor("out_p", [C, S], f32, kind="ExternalOutput").ap()
    out_cf = nc.dram_tensor("out_cf", [128, S], f32,
                            kind="ExternalOutput").ap()

    with tile.TileContext(nc) as tc, ExitStack() as ctx:
        main = ctx.enter_context(tc.tile_pool(name="main", bufs=1))

        g_sb = main.tile([128, 1], f32, tag="g")
        ebias = main.tile([128, 1], f32, tag="eb")
        nc.gpsimd.memset(ebias[:], EXP_BIAS)

        # ---- persistent SBUF tiles (one merged DMA per DRAM tensor) ----
        comp16 = main.tile([128, 4 * S], bf16, tag="cm16", name="cm16")
        ctok16 = main.tile([128, 4 * S], bf16, tag="ct16", name="ct16")
        cf16 = main.tile([128, 4 * S], bf16, tag="cf16", name="cf16")
        wq16 = main.tile([128, 4 * C], bf16, tag="wq16", name="wq16")
        wkv16 = main.tile([128, 4 * 2 * C], bf16, tag="wkv16", name="wkv16")
        wp16 = main.tile([128, 4 * C], bf16, tag="wp16", name="wp16")
        wcs16 = main.tile([128, 4 * C], bf16, tag="wcs16", name="wcs16")
        wcc16 = main.tile([128, 4 * 128], bf16, tag="wcc16", name="wcc16")

        comp_c = [comp16[:, S * k:S * (k + 1)] for k in range(4)]
        ctok_c = [ctok16[:, S * k:S * (k + 1)] for k in range(4)]
        cf_c = [cf16[:, S * k:S * (k + 1)] for k in range(4)]
        wq_c = [wq16[:, C * k:C * (k + 1)] for k in range(4)]
        wkv_c = [wkv16[:, 2 * C * k:2 * C * (k + 1)] for k in range(4)]
        wp_c = [wp16[:, C * k:C * (k + 1)] for k in range(4)]
        wcs_c = [wcs16[:, C * k:C * (k + 1)] for k in range(4)]
        wcc_c = [wcc16[:, 128 * k:128 * (k + 1)] for k in range(4)]

        kT16 = [main.tile([128, S], bf16, tag=f"kt{j}", name=f"kt{j}")
                for j in range(4)]
        qT16 = [main.tile([128, S], bf16, tag=f"qt{j}", name=f"qt{j}")
                for j in range(4)]
        # v pair tiles: [128 keys, (t2=2, h=8, e=128)] fp8e4; e=0 is the 1.0
        # ones column (Z lands on o row 0), e=64:128 the v data, 1:64 zeros.
        vp = [main.tile([128, 2 * 8 * 128], fp8, tag=f"vp{p}", name=f"vp{p}")
              for p in range(4)]
        rtb = [main.tile([128, S], bf16, tag=f"rt{j}", name=f"rt{j}")
               for j in range(4)]
        outcf_sb = main.tile([128, S], f32, tag="ocf", name="ocf")

        # ---- DMA emission order: attention-critical first ----
        def dma_merged(dst_tile, src_ap, k, lo=0, hi=None):
            hi = k if hi is None else hi
            src3 = src_ap.rearrange("(k p) s -> p k s", k=k)
            dst3 = dst_tile[:].rearrange("p (k s) -> p k s", k=k)
            nc.sync.dma_start(dst3[:, lo:hi, :], src3[:, lo:hi, :])

        dma_merged(comp16, comp16d, 4, 0, 2)
        dma_merged(wkv16, wkv16d, 4, 0, 2)
        dma_merged(comp16, comp16d, 4, 2, 4)
        dma_merged(wkv16, wkv16d, 4, 2, 4)
        dma_merged(ctok16, ctok16d, 4)
        dma_merged(wq16, wq16d, 4)
        nc.sync.dma_start(g_sb[:], gated[:])
        dma_merged(cf16, cf16d, 4)
        dma_merged(wcc16, wcc16d, 4)
        dma_merged(wp16, wp16d, 4)
        dma_merged(wcs16, wcs16d, 4)

        # ones + zero-pad columns of the v tiles (SBUF memset = Pool)
        for p in range(4):
            vv = vp[p][:].rearrange("p (t h e) -> p t h e", t=2, h=8)
            nc.gpsimd.memset(vv[:, :, :, 0:1], 1.0)
            nc.gpsimd.memset(vv[:, :, :, 1:64], 0.0)

        warm_src = main.tile([128, 128], bf16, tag="warm", name="warm")
        nc.gpsimd.memset(warm_src[:], 0.25)

        with tc.tile_pool(name="psMain", bufs=1, space="PSUM") as ps:
            # p-state spin-up during the input DMA wait
            for _ in range(12):
                wtp = ps.tile([128, 1024], f32, tag="sc", bufs=3)
                nc.tensor.matmul(wtp[:, 0:128], warm_src[:], warm_src[:],
                                 start=True, stop=True)

            # ---- QKV projection half-jobs (bf16, ~2048 PE cycles each) ----
            kq_acc = {}

            def emit_kq_half(dst, w_c_, x_c_, j, t):
                if (id(dst), j) not in kq_acc:
                    kq_acc[(id(dst), j)] = ps.tile(
                        [128, 1024], f32, tag="sc", bufs=3, name=f"kqa{j}")
                acc = kq_acc[(id(dst), j)]
                for k in range(4):
                    nc.tensor.matmul(
                        acc[:, 512 * t:512 * (t + 1)],
                        w_c_[k][:, 128 * j:128 * (j + 1)],
                        x_c_[k][:, 512 * t:512 * (t + 1)],
                        start=(k == 0), stop=(k == 3))
                if t == 1:
                    del kq_acc[(id(dst), j)]
                    nc.vector.tensor_copy(dst[:], acc[:])

            v_acc = {}

            def emit_v_half(p, t2):
                # token block 2p+t2; drain (strided, ->fp8e4) at t2==1
                if p not in v_acc:
                    v_acc[p] = ps.tile([128, 1024], f32, tag="sc", bufs=3,
                                       name=f"va{p}")
                acc = v_acc[p]
                tb = 2 * p + t2
                for k in range(4):
                    nc.tensor.matmul(
                        acc[:, 512 * t2:512 * (t2 + 1)],
                        comp_c[k][:, 128 * tb:128 * (tb + 1)],
                        wkv_c[k][:, C:2 * C],
                        start=(k == 0), stop=(k == 3))
                if t2 == 1:
                    del v_acc[p]
                    dst = vp[p][:].rearrange("p (t h e) -> p t h e", t=2, h=8)
                    src = acc[:].rearrange("p (t h d) -> p t h d", t=2, h=8)
                    nc.vector.tensor_copy(dst[:, :, :, 64:128],
                                          src[:, :, :, :])

            def emit_conv_cf(pc):
                acc = ps.tile([128, 1024], f32, tag="sc", bufs=3)
                for g in range(4):
                    nc.tensor.matmul(acc[:, 0:512], wcc_c[g][:, :],
                                     cf_c[g][:, 512 * pc:512 * (pc + 1)],
                                     start=(g == 0), stop=(g == 3))
                nc.vector.tensor_copy(
                    outcf_sb[:, 512 * pc:512 * (pc + 1)], acc[:, 0:512])
                nc.sync.dma_start(
                    out_cf[:, 512 * pc:512 * (pc + 1)],
                    outcf_sb[:, 512 * pc:512 * (pc + 1)])

            # pre-phase: j0 of K and Q so head 0 can start immediately
            emit_kq_half(kT16[0], wkv_c, comp_c, 0, 0)
            emit_kq_half(kT16[0], wkv_c, comp_c, 0, 1)
            emit_kq_half(qT16[0], wq_c, ctok_c, 0, 0)
            emit_kq_half(qT16[0], wq_c, ctok_c, 0, 1)

            # interleaved half-jobs, keyed by the (head, kt) slot AFTER whose
            # score-matmuls they are emitted.  v pair p is needed by the o
            # matmul at slot (h,6)/(h,7)/(h+1,0)/(h+1,1); kT/qT j by (2j,0).
            ilv = {
                (0, 1): lambda: emit_v_half(0, 0),
                (0, 2): lambda: emit_v_half(0, 1),
                (0, 3): lambda: emit_v_half(1, 0),
                (0, 4): lambda: emit_v_half(1, 1),
                (0, 5): lambda: emit_v_half(2, 0),
                (0, 6): lambda: emit_v_half(2, 1),
                (0, 7): lambda: emit_v_half(3, 0),
                (1, 0): lambda: emit_v_half(3, 1),
                (1, 2): lambda: emit_kq_half(kT16[1], wkv_c, comp_c, 1, 0),
                (1, 3): lambda: emit_kq_half(kT16[1], wkv_c, comp_c, 1, 1),
                (1, 4): lambda: emit_kq_half(qT16[1], wq_c, ctok_c, 1, 0),
                (1, 5): lambda: emit_kq_half(qT16[1], wq_c, ctok_c, 1, 1),
                (2, 2): lambda: emit_kq_half(kT16[2], wkv_c, comp_c, 2, 0),
                (2, 3): lambda: emit_kq_half(kT16[2], wkv_c, comp_c, 2, 1),
                (2, 4): lambda: emit_kq_half(qT16[2], wq_c, ctok_c, 2, 0),
                (2, 5): lambda: emit_kq_half(qT16[2], wq_c, ctok_c, 2, 1),
                (3, 2): lambda: emit_kq_half(kT16[3], wkv_c, comp_c, 3, 0),
                (3, 3): lambda: emit_kq_half(kT16[3], wkv_c, comp_c, 3, 1),
                (3, 4): lambda: emit_kq_half(qT16[3], wq_c, ctok_c, 3, 0),
                (3, 5): lambda: emit_kq_half(qT16[3], wq_c, ctok_c, 3, 1),
                (4, 2): lambda: emit_conv_cf(0),
                (4, 4): lambda: emit_conv_cf(1),
            }

            # ---- attention ----
            pt_pool = {}      # (h, p) -> pt pair tile [128, 2048] e5m2
            o_tiles = {}      # h -> o psum tile

            def emit_sc(h, kt):
                jq, row = h // 2, 64 * (h % 2)
                sc = ps.tile([128, S], f32, tag="sc", bufs=3)
                for qc in range(2):
                    nc.tensor.matmul(
                        sc[:, 512 * qc:512 * (qc + 1)],
                        kT16[jq][row:row + 64, 128 * kt:128 * (kt + 1)],
                        qT16[jq][row:row + 64, 512 * qc:512 * (qc + 1)],
                        start=True, stop=True)
                if kt % 2 == 0:
                    pt_pool[(h, kt // 2)] = main.tile(
                        [128, 2048], fp8e5, tag="pt", bufs=6,
                        name=f"pt{h}_{kt // 2}")
                pt = pt_pool[(h, kt // 2)]
                nc.scalar.activation(pt[:, 1024 * (kt % 2):1024 * (kt % 2 + 1)],
                                     sc[:], EXP, bias=ebias[:, 0:1],
                                     scale=SCALE)

            def emit_o(h, p):
                # o[0,:] = Z, o[64:128,:] = P@v_h  (DoubleRow over kt pair)
                if h not in o_tiles:
                    o_tiles[h] = ps.tile([128, S], f32, tag="o", bufs=1,
                                         name=f"o{h}")
                o_ps = o_tiles[h]
                vv = vp[p][:].rearrange("p (t h e) -> p t h e", t=2, h=8)
                ptv = pt_pool.pop((h, p))[:].rearrange("p (t q) -> p t q", t=2)
                for qc in range(2):
                    nc.tensor.matmul(
                        o_ps[:, 512 * qc:512 * (qc + 1)],
                        vv[:, :, h:h + 1, :],
                        ptv[:, :, 512 * qc:512 * (qc + 1)],
                        start=(p == 0), stop=(p == 3), perf_mode=DR)

            def emit_norm(h):
                jq, row = h // 2, 64 * (h % 2)
                o_ps = o_tiles.pop(h)
                zi = main.tile([1, S], f32, tag="zi", bufs=2, name=f"zi{h}")
                zbc = main.tile([64, S], f32, tag="zb", bufs=2, name=f"zb{h}")
                nc.vector.reciprocal_approx_fast(zi[0:1, :], o_ps[0:1, :])
                nc.gpsimd.partition_broadcast(zbc[0:64, :], zi[0:1, :])
                nc.vector.scalar_tensor_tensor(
                    rtb[jq][row:row + 64, :], o_ps[64:128, :], 1.0,
                    zbc[0:64, :], BYPASS, MULT)
                if h % 2 == 1:  # pair complete: s_in^T = rtb + gate*ctokT
                    nc.vector.scalar_tensor_tensor(
                        rtb[jq][:], ctok_c[jq][:], g_sb[:, 0:1], rtb[jq][:],
                        MULT, ADD)

            # flat emission; o(h) trails into head h+1 per the o-bank cycle
            for h in range(NH):
                for kt in range(8):
                    emit_sc(h, kt)
                    if h > 0:
                        if kt == 0:
                            emit_o(h - 1, 2)
                        elif kt == 1:
                            emit_o(h - 1, 3)
                        elif kt == 2:
                            emit_norm(h - 1)
                    if (h, kt) in ilv:
                        ilv[(h, kt)]()
                    if kt == 6:
                        emit_o(h, 0)
                    elif kt == 7:
                        emit_o(h, 1)
            emit_o(NH - 1, 2)
            emit_o(NH - 1, 3)
            emit_norm(NH - 1)

        # ---- tail: proj (s2d-layout direct) + conv ----
        # proj out (par, g): rows = s2d channels 128g.., cols = spatial half
        # par; lhsT = stride-2 column slices of rtb[j], rhs = Wproj chunk j.
        rtb_v = [rtb[j][:].rearrange("p (g i two) -> p g two i", g=4, two=2)
                 for j in range(4)]
        s2d_sb = [[main.tile([128, 512], bf16, tag=f"s2d{par}{g}",
                             name=f"s2d{par}{g}") for g in range(4)]
                  for par in range(2)]
        ost = [main.tile([128, 512], f32, tag=f"ost{i}", bufs=2,
                         name=f"ost{i}") for i in range(2)]

        with tc.tile_pool(name="psTail", bufs=1, space="PSUM") as psT:
            held = {}

            def emit_proj_pre(par, g):
                acc = psT.tile([128, 512], f32, tag="pj", bufs=8)
                for j in range(3):
                    nc.tensor.matmul(acc[:], rtb_v[j][:, g, par, :],
                                     wp_c[j][:, :], start=(j == 0), stop=False)
                held[(par, g)] = acc

            def emit_proj_fin(par, g, eng):
                acc = held.pop((par, g))
                nc.tensor.matmul(acc[:], rtb_v[3][:, g, par, :],
                                 wp_c[3][:, :], start=False, stop=True)
                eng(s2d_sb[par][g][:], acc[:])

            def emit_conv_s(par):
                for oc in range(4):
                    acc = psT.tile([128, 512], f32, tag="pj", bufs=8)
                    for g in range(4):
                        nc.tensor.matmul(acc[:],
                                         wcs_c[g][:, 128 * oc:128 * (oc + 1)],
                                         s2d_sb[par][g][:],
                                         start=(g == 0), stop=(g == 3))
                    o_sb = ost[oc % 2]
                    eng = (nc.vector.tensor_copy if oc % 2 == 0
                           else nc.scalar.copy)
                    eng(o_sb[:], acc[:])
                    nc.sync.dma_start(
                        out_p[128 * oc:128 * (oc + 1),
                              512 * par:512 * (par + 1)], o_sb[:])

            # pre-start j0..2 (needs only rtb0-2; banks free as the last
            # head's exps drain the sc pool)
            for g in range(4):
                emit_proj_pre(0, g)
            for g in range(4):
                emit_proj_pre(1, g)
            for g in range(4):
                emit_proj_fin(0, g, nc.vector.tensor_copy if g % 2 == 0
                              else nc.scalar.copy)
            for g in range(4):
                emit_proj_fin(1, g, nc.vector.tensor_copy if g % 2 == 0
                              else nc.scalar.copy)
            emit_conv_s(0)
            emit_conv_s(1)

    nc.compile()
    _CACHE["nc"] = nc
    return nc


def _shard_inputs(content_feat, components, pos_emb, Wq, Wkv, Wproj, bproj,
                  Wconv, bconv):
    import ml_dtypes

    bf = ml_dtypes.bfloat16
    f = np.float32
    pos2 = np.asarray(pos_emb, dtype=f).reshape(S, C)
    wq16 = np.asarray(Wq, dtype=f).astype(bf)
    wkv16 = np.asarray(Wkv, dtype=f).astype(bf)
    wp16 = np.asarray(Wproj, dtype=f).astype(bf)
    wcT = np.ascontiguousarray(np.asarray(Wconv, dtype=f).T)
    wcs16 = np.ascontiguousarray(wcT[:C]).astype(bf)
    in_maps = []
    for core in range(N_CORES):
        b, n = core // 4, core % 4
        ctokT = np.ascontiguousarray(
            (np.asarray(content_feat[b], dtype=f).reshape(S, C) + pos2).T)
        compT = np.ascontiguousarray(
            (np.asarray(components[n, b], dtype=f).reshape(S, C) + pos2).T)
        in_maps.append({
            "comp16": compT.astype(bf),
            "ctok16": ctokT.astype(bf),
            "cf16": np.ascontiguousarray(
                np.asarray(content_feat[b], dtype=f).reshape(C, S)).astype(bf),
            "wq16": wq16,
            "wkv16": wkv16,
            "wp16": wp16,
            "wcs16": wcs16,
            "wcc16": np.ascontiguousarray(
                wcT[C:, 128 * n:128 * (n + 1)]).astype(bf),
            "gate": np.full((128, 1), 1.0 if n == 0 else 0.0, dtype=f),
        })
    return in_maps


def _run(trace=False, **inputs):
    from concourse.bass_utils import run_bass_kernel_spmd

    nc = _build()
    in_maps = _shard_inputs(**inputs)
    res = run_bass_kernel_spmd(nc, in_maps, list(range(N_CORES)), trace=trace)
    outs = [np.asarray(res.results[i]["out_p"], dtype=np.float64)
            for i in range(N_CORES)]
    out = np.stack([outs[0] + outs[1] + outs[2] + outs[3],
                    outs[4] + outs[5] + outs[6] + outs[7]], axis=0)
    for core in range(N_CORES):
        b, n = core // 4, core % 4
        out[b, 128 * n:128 * (n + 1), :] += np.asarray(
            res.results[core]["out_cf"], dtype=np.float64)
    # host-side affine constants: out += bconv[o] + ws[o]*bproj[i%512]
    # with ws[o] = sum_c Wconv[o, c<C]  (bproj enters via the conv s-half)
    Wconv = np.asarray(inputs["Wconv"], dtype=np.float64)
    bproj = np.asarray(inputs["bproj"], dtype=np.float64)
    bconv = np.asarray(inputs["bconv"], dtype=np.float64)
    ws = Wconv[:, :C].sum(axis=1)
    bias_img = bconv[:, None] + np.outer(ws, np.concatenate([bproj, bproj]))
    out += bias_img[None, :, :]
    return out.reshape(B, C, H, W).astype(np.float32), res


def kernel(**inputs):
    out, _ = _run(trace=False, **inputs)
    return out


# revision 16
# speedup vs baseline: 1.0719x; 1.0719x over previous
"""Trainium2 Bass kernel for nn_Attention_54391465836966.

Math (per batch b, component n; one (b,n) pair per core, host sums over n):
  ctok = content_feat[b].raw_reshape(S,C) + pos          # [1024, 512]
  comp_tok = components[n,b].raw_reshape(S,C) + pos
  q = ctok @ Wq ; k,v = comp_tok @ Wkv (split)
  per head h: P = exp(scale q_h k_h^T); o_h = (P @ v_h) / rowsum(P)
  s_in = concat_h(o_h) + gate*ctok                        # gate = [n==0]
  s2d  = raw_reshape(s_in @ Wproj)                        # bias via host
  out  = Wconv^T[:C] @ s2d + Wconv^T[C:] @ cf             # + bias via host

Design notes (cost-model-driven):
- The P@V (o) matmuls run fp8 DoubleRow (2 k-tiles per instruction at 0.5
  cycles/row = 4x the bf16 rate): pt = exp output in e5m2 (its 22-efold
  dynamic range covers the unnormalized exp spread -- score sigma ~2.0 ->
  +-11 efolds; e4m3's 11.7 efolds would overflow), v drained to e4m3.
  Attention-output noise from fp8 is diluted by the ctok residual and
  P-quantization partially cancels against Z (measured 1.4e-2 total).
- The q/k path stays bf16 end-to-end: fp8 anywhere before the exp adds
  ~0.17 absolute score noise -> 17% P-reweighting with only ~19 effective
  keys after softmax (score sigma 2.0) -> 3-4e-2 output error (measured).
- exp writes pt e5m2 directly with a constant -4 bias (normalization
  cancels it exactly; caps pt at exp(score-4) with overflow only past
  7.5 sigma scores, flush below -7.1 negligible).
- Z rides row 0 of the o PSUM tile (ones column first in v): the custom-DVE
  reciprocal reads partition 0 of PSUM directly (hw quirk: it ignores the
  AP's partition base), skipping the per-head z-copy entirely.  Drain +
  normalize fuse into one scalar_tensor_tensor from PSUM (v data sits at
  e=64:128 so the o data rows land at the 32-aligned partition base 64).
- The projection uses stride-2 column slices of s_in^T as the stationary
  operand, which makes its PSUM output land directly in the s->s2d
  raw-reshape layout: no PE transposes anywhere in the kernel.
- bproj/bconv are affine constants independent of the data path; their
  contribution is bconv[o] + (sum_c Wconv[o,c<C]) * bproj[i%512], a rank-1
  image the host adds during unshard.
- PSUM: score tiles tag "sc" [128,1024] bufs=3 (6 banks) shared by the
  QKV-projection accumulations (pre-phase + interleaved jobs borrow
  rotation slots); o tag [128,1024] bufs=1 (2 banks).  GPSIMD cannot touch
  PSUM, so all PSUM drains go to DVE (ACT helps only post-exp in the tail).
- ACT runs exps only (64 x [128,1024] ~= 1.04us each); QKV/conv_cf jobs are
  split into ~2048-cycle half-jobs and interleaved between score matmuls so
  the exp stream never starves.  o matmuls for head h trail into head h+1's
  first slots; the recip->broadcast->normalize chain then frees the single
  o buffer before head h+1 needs it.
"""
import sys

sys.path.insert(0, "/opt/trn_rl_repo")

import numpy as np

N_CORES = 8
B, C, H, W = 2, 512, 32, 32
S = H * W  # 1024
NH, HD = 8, 64
SCALE = HD ** -0.5
EXP_BIAS = -4.0

_CACHE = {}


def _build():
    if "nc" in _CACHE:
        return _CACHE["nc"]
    from contextlib import ExitStack

    import concourse.bacc as bacc
    import concourse.mybir as mybir
    import concourse.tile as tile

    f32 = mybir.dt.float32
    bf16 = mybir.dt.bfloat16
    fp8 = mybir.dt.float8e4
    fp8e5 = mybir.dt.float8e5
    EXP = mybir.ActivationFunctionType.Exp
    MULT = mybir.AluOpType.mult
    ADD = mybir.AluOpType.add
    BYPASS = mybir.AluOpType.bypass
    DR = mybir.MatmulPerfMode.DoubleRow

    nc = bacc.Bacc("TRN2", target_bir_lowering=False, debug=False,
                   num_devices=N_CORES)

    din = lambda n, s, dt: nc.dram_tensor(n, s, dt, kind="ExternalInput").ap()
    comp16d = din("comp16", [C, S], bf16)   # (comp_tok + pos).T
    ctok16d = din("ctok16", [C, S], bf16)   # (content_tok + pos).T
    cf16d = din("cf16", [C, S], bf16)       # content_feat[b] raw [C,S]
    wq16d = din("wq16", [C, C], bf16)       # Wq
    wkv16d = din("wkv16", [C, 2 * C], bf16)  # Wkv (cols 0:C K, C:2C V)
    wp16d = din("wp16", [C, C], bf16)       # Wproj
    wcs16d = din("wcs16", [C, C], bf16)     # Wconv.T rows :C (s half)
    wcc16d = din("wcc16", [C, 128], bf16)   # Wconv.T[C:, 128n:128(n+1)]
    gated = din("gate", [128, 1], f32)      # 1.0 on n==0 cores else 0.0
    out_p = nc.dram_tensor("out_p", [C, S], bf16, kind="ExternalOutput").ap()
    out_cf = nc.dram_tensor("out_cf", [128, S], bf16,
                            kind="ExternalOutput").ap()

    with tile.TileContext(nc) as tc, ExitStack() as ctx:
        main = ctx.enter_context(tc.tile_pool(name="main", bufs=1))

        g_sb = main.tile([128, 1], f32, tag="g")
        ebias = main.tile([128, 1], f32, tag="eb")
        nc.gpsimd.memset(ebias[:], EXP_BIAS)

        # ---- persistent SBUF tiles (one merged DMA per DRAM tensor) ----
        comp16 = main.tile([128, 4 * S], bf16, tag="cm16", name="cm16")
        ctok16 = main.tile([128, 4 * S], bf16, tag="ct16", name="ct16")
        cf16 = main.tile([128, 4 * S], bf16, tag="cf16", name="cf16")
        wq16 = main.tile([128, 4 * C], bf16, tag="wq16", name="wq16")
        wkv16 = main.tile([128, 4 * 2 * C], bf16, tag="wkv16", name="wkv16")
        wp16 = main.tile([128, 4 * C], bf16, tag="wp16", name="wp16")
        wcs16 = main.tile([128, 4 * C], bf16, tag="wcs16", name="wcs16")
        wcc16 = main.tile([128, 4 * 128], bf16, tag="wcc16", name="wcc16")

        comp_c = [comp16[:, S * k:S * (k + 1)] for k in range(4)]
        ctok_c = [ctok16[:, S * k:S * (k + 1)] for k in range(4)]
        cf_c = [cf16[:, S * k:S * (k + 1)] for k in range(4)]
        wq_c = [wq16[:, C * k:C * (k + 1)] for k in range(4)]
        wkv_c = [wkv16[:, 2 * C * k:2 * C * (k + 1)] for k in range(4)]
        wp_c = [wp16[:, C * k:C * (k + 1)] for k in range(4)]
        wcs_c = [wcs16[:, C * k:C * (k + 1)] for k in range(4)]
        wcc_c = [wcc16[:, 128 * k:128 * (k + 1)] for k in range(4)]

        kT16 = [main.tile([128, S], bf16, tag=f"kt{j}", name=f"kt{j}")
                for j in range(4)]
        qT16 = [main.tile([128, S], bf16, tag=f"qt{j}", name=f"qt{j}")
                for j in range(4)]
        # v pair tiles: [128 keys, (t2=2, h=8, e=128)] fp8e4; e=0 is the 1.0
        # ones column (Z lands on o row 0), e=64:128 the v data, 1:64 zeros.
        vp = [main.tile([128, 2 * 8 * 128], fp8, tag=f"vp{p}", name=f"vp{p}")
              for p in range(4)]
        rtb = [main.tile([128, S], bf16, tag=f"rt{j}", name=f"rt{j}")
               for j in range(4)]
        outcf_sb = main.tile([128, S], bf16, tag="ocf", name="ocf")

        # ---- DMA emission order: attention-critical first ----
        def dma_merged(dst_tile, src_ap, k, lo=0, hi=None):
            hi = k if hi is None else hi
            src3 = src_ap.rearrange("(k p) s -> p k s", k=k)
            dst3 = dst_tile[:].rearrange("p (k s) -> p k s", k=k)
            nc.sync.dma_start(dst3[:, lo:hi, :], src3[:, lo:hi, :])

        def dma_cols(dst_tile, src_ap, k, c0, c1):
            # column-group slice across all k row-chunks in one strided DMA
            src3 = src_ap.rearrange("(k p) s -> p k s", k=k)
            dst3 = dst_tile[:].rearrange("p (k s) -> p k s", k=k)
            nc.sync.dma_start(dst3[:, :, c0:c1], src3[:, :, c0:c1])

        # critical path for the first score matmul: comp + ctok + the j0
        # column groups of Wk/Wq (~2.25MB); everything else after
        dma_merged(comp16, comp16d, 4, 0, 2)
        dma_cols(wkv16, wkv16d, 4, 0, 128)        # Wk j0 cols
        dma_merged(comp16, comp16d, 4, 2, 4)
        dma_merged(ctok16, ctok16d, 4, 0, 2)
        dma_cols(wq16, wq16d, 4, 0, 128)          # Wq j0 cols
        dma_merged(ctok16, ctok16d, 4, 2, 4)
        nc.sync.dma_start(g_sb[:], gated[:])
        dma_cols(wkv16, wkv16d, 4, 512, 1024)     # V half (v jobs, early)
        dma_cols(wkv16, wkv16d, 4, 128, 512)      # Wk j1-3
        dma_cols(wq16, wq16d, 4, 128, 512)        # Wq j1-3
        dma_merged(cf16, cf16d, 4)
        dma_merged(wcc16, wcc16d, 4)
        dma_merged(wp16, wp16d, 4)
        dma_merged(wcs16, wcs16d, 4)

        # ones + zero-pad columns of the v tiles (SBUF memset = Pool)
        for p in range(4):
            vv = vp[p][:].rearrange("p (t h e) -> p t h e", t=2, h=8)
            nc.gpsimd.memset(vv[:, :, :, 0:1], 1.0)
            nc.gpsimd.memset(vv[:, :, :, 1:64], 0.0)

        warm_src = main.tile([128, 128], bf16, tag="warm", name="warm")
        nc.gpsimd.memset(warm_src[:], 0.25)
        # preload the Exp activation table (1.28us) off the critical stream
        dummy_pt = main.tile([1, 8], fp8e5, tag="dpt", name="dpt")
        nc.scalar.activation(dummy_pt[0:1, :], warm_src[0:1, 0:8], EXP,
                             bias=ebias[0:1, 0:1], scale=SCALE)

        with tc.tile_pool(name="psMain", bufs=1, space="PSUM") as ps:
            # p-state spin-up gated on the first comp chunk so the ramp is
            # still warm when the first kq matmuls run (dep-free warms would
            # finish during the DMA wait and let the clock reset)
            for _ in range(12):
                wtp = ps.tile([128, 1024], f32, tag="sc", bufs=3)
                nc.tensor.matmul(wtp[:, 0:128], warm_src[:],
                                 comp16[:, 0:128], start=True, stop=True)

            # ---- QKV projection half-jobs (bf16, ~2048 PE cycles each) ----
            kq_acc = {}

            def emit_kq_half(dst, w_c_, x_c_, j, t):
                if (id(dst), j) not in kq_acc:
                    kq_acc[(id(dst), j)] = ps.tile(
                        [128, 1024], f32, tag="sc", bufs=3, name=f"kqa{j}")
                acc = kq_acc[(id(dst), j)]
                for k in range(4):
                    nc.tensor.matmul(
                        acc[:, 512 * t:512 * (t + 1)],
                        w_c_[k][:, 128 * j:128 * (j + 1)],
                        x_c_[k][:, 512 * t:512 * (t + 1)],
                        start=(k == 0), stop=(k == 3))
                if t == 1:
                    del kq_acc[(id(dst), j)]
                    nc.vector.tensor_copy(dst[:], acc[:])

            v_acc = {}

            def emit_v_half(p, t2):
                # token block 2p+t2; drain (strided, ->fp8e4) at t2==1
                if p not in v_acc:
                    v_acc[p] = ps.tile([128, 1024], f32, tag="sc", bufs=3,
                                       name=f"va{p}")
                acc = v_acc[p]
                tb = 2 * p + t2
                for k in range(4):
                    nc.tensor.matmul(
                        acc[:, 512 * t2:512 * (t2 + 1)],
                        comp_c[k][:, 128 * tb:128 * (tb + 1)],
                        wkv_c[k][:, C:2 * C],
                        start=(k == 0), stop=(k == 3))
                if t2 == 1:
                    del v_acc[p]
                    dst = vp[p][:].rearrange("p (t h e) -> p t h e", t=2, h=8)
                    src = acc[:].rearrange("p (t h d) -> p t h d", t=2, h=8)
                    nc.vector.tensor_copy(dst[:, :, :, 64:128],
                                          src[:, :, :, :])

            def emit_conv_cf(pc):
                acc = ps.tile([128, 1024], f32, tag="sc", bufs=3)
                for g in range(4):
                    nc.tensor.matmul(acc[:, 0:512], wcc_c[g][:, :],
                                     cf_c[g][:, 512 * pc:512 * (pc + 1)],
                                     start=(g == 0), stop=(g == 3))
                nc.vector.tensor_copy(
                    outcf_sb[:, 512 * pc:512 * (pc + 1)], acc[:, 0:512])
                nc.sync.dma_start(
                    out_cf[:, 512 * pc:512 * (pc + 1)],
                    outcf_sb[:, 512 * pc:512 * (pc + 1)])

            # pre-phase: j0 of K and Q so head 0 can start immediately
            emit_kq_half(kT16[0], wkv_c, comp_c, 0, 0)
            emit_kq_half(kT16[0], wkv_c, comp_c, 0, 1)
            emit_kq_half(qT16[0], wq_c, ctok_c, 0, 0)
            emit_kq_half(qT16[0], wq_c, ctok_c, 0, 1)

            # interleaved half-jobs, keyed by the (head, kt) slot AFTER whose
            # score-matmuls they are emitted.  v pair p is needed by the o
            # matmul at slot (h,6)/(h,7)/(h+1,0)/(h+1,1); kT/qT j by (2j,0).
            ilv = {
                (0, 1): lambda: emit_v_half(0, 0),
                (0, 2): lambda: emit_v_half(0, 1),
                (0, 3): lambda: emit_v_half(1, 0),
                (0, 4): lambda: emit_v_half(1, 1),
                (0, 5): lambda: emit_v_half(2, 0),
                (0, 6): lambda: emit_v_half(2, 1),
                (0, 7): lambda: emit_v_half(3, 0),
                (1, 0): lambda: emit_v_half(3, 1),
                (1, 2): lambda: emit_kq_half(kT16[1], wkv_c, comp_c, 1, 0),
                (1, 3): lambda: emit_kq_half(kT16[1], wkv_c, comp_c, 1, 1),
                (1, 4): lambda: emit_kq_half(qT16[1], wq_c, ctok_c, 1, 0),
                (1, 5): lambda: emit_kq_half(qT16[1], wq_c, ctok_c, 1, 1),
                (2, 2): lambda: emit_kq_half(kT16[2], wkv_c, comp_c, 2, 0),
                (2, 3): lambda: emit_kq_half(kT16[2], wkv_c, comp_c, 2, 1),
                (2, 4): lambda: emit_kq_half(qT16[2], wq_c, ctok_c, 2, 0),
                (2, 5): lambda: emit_kq_half(qT16[2], wq_c, ctok_c, 2, 1),
                (3, 2): lambda: emit_kq_half(kT16[3], wkv_c, comp_c, 3, 0),
                (3, 3): lambda: emit_kq_half(kT16[3], wkv_c, comp_c, 3, 1),
                (3, 4): lambda: emit_kq_half(qT16[3], wq_c, ctok_c, 3, 0),
                (3, 5): lambda: emit_kq_half(qT16[3], wq_c, ctok_c, 3, 1),
                (7, 2): lambda: emit_conv_cf(0),
                (7, 4): lambda: emit_conv_cf(1),
            }

            # ---- attention ----
            pt_pool = {}      # (h, p) -> pt pair tile [128, 2048] e5m2
            o_tiles = {}      # h -> o psum tile

            def emit_sc(h, kt):
                jq, row = h // 2, 64 * (h % 2)
                sc = ps.tile([128, S], f32, tag="sc", bufs=3)
                for qc in range(2):
                    nc.tensor.matmul(
                        sc[:, 512 * qc:512 * (qc + 1)],
                        kT16[jq][row:row + 64, 128 * kt:128 * (kt + 1)],
                        qT16[jq][row:row + 64, 512 * qc:512 * (qc + 1)],
                        start=True, stop=True)
                if kt % 2 == 0:
                    pt_pool[(h, kt // 2)] = main.tile(
                        [128, 2048], fp8e5, tag="pt", bufs=6,
                        name=f"pt{h}_{kt // 2}")
                pt = pt_pool[(h, kt // 2)]
                nc.scalar.activation(pt[:, 1024 * (kt % 2):1024 * (kt % 2 + 1)],
                                     sc[:], EXP, bias=ebias[:, 0:1],
                                     scale=SCALE)

            def emit_o(h, p):
                # o[0,:] = Z, o[64:128,:] = P@v_h  (DoubleRow over kt pair)
                if h not in o_tiles:
                    o_tiles[h] = ps.tile([128, S], f32, tag="o", bufs=1,
                                         name=f"o{h}")
                o_ps = o_tiles[h]
                vv = vp[p][:].rearrange("p (t h e) -> p t h e", t=2, h=8)
                ptv = pt_pool.pop((h, p))[:].rearrange("p (t q) -> p t q", t=2)
                for qc in range(2):
                    nc.tensor.matmul(
                        o_ps[:, 512 * qc:512 * (qc + 1)],
                        vv[:, :, h:h + 1, :],
                        ptv[:, :, 512 * qc:512 * (qc + 1)],
                        start=(p == 0), stop=(p == 3), perf_mode=DR)

            def emit_norm(h):
                jq, row = h // 2, 64 * (h % 2)
                o_ps = o_tiles.pop(h)
                zi = main.tile([1, S], f32, tag="zi", bufs=2, name=f"zi{h}")
                zbc = main.tile([64, S], f32, tag="zb", bufs=2, name=f"zb{h}")
                nc.vector.reciprocal_approx_fast(zi[0:1, :], o_ps[0:1, :])
                nc.gpsimd.partition_broadcast(zbc[0:64, :], zi[0:1, :])
                nc.vector.scalar_tensor_tensor(
                    rtb[jq][row:row + 64, :], o_ps[64:128, :], 1.0,
                    zbc[0:64, :], BYPASS, MULT)
                if h % 2 == 1:  # pair complete: s_in^T = rtb + gate*ctokT
                    nc.vector.scalar_tensor_tensor(
                        rtb[jq][:], ctok_c[jq][:], g_sb[:, 0:1], rtb[jq][:],
                        MULT, ADD)

            # flat emission; o(h) trails into head h+1 per the o-bank cycle
            for h in range(NH):
                for kt in range(8):
                    emit_sc(h, kt)
                    if h > 0:
                        if kt == 0:
                            emit_o(h - 1, 2)
                        elif kt == 1:
                            emit_o(h - 1, 3)
                        elif kt == 2:
                            emit_norm(h - 1)
                    if (h, kt) in ilv:
                        ilv[(h, kt)]()
                    if kt == 6:
                        emit_o(h, 0)
                    elif kt == 7:
                        emit_o(h, 1)
            def warm_fill(n):
                for _ in range(n):
                    wtp = ps.tile([128, 1024], f32, tag="sc", bufs=3)
                    nc.tensor.matmul(wtp[:, 0:512], warm_src[:],
                                     comp16[:, 0:512], start=True, stop=True)

            emit_o(NH - 1, 2)
            warm_fill(2)
            emit_o(NH - 1, 3)
            emit_norm(NH - 1)
            # keep the PE clock hot through the last normalization chain so
            # the projection/conv tail runs at full p-state
            warm_fill(6)

        # ---- tail: proj (s2d-layout direct) + conv ----
        # proj out (par, g): rows = s2d channels 128g.., cols = spatial half
        # par; lhsT = stride-2 column slices of rtb[j], rhs = Wproj chunk j.
        rtb_v = [rtb[j][:].rearrange("p (g i two) -> p g two i", g=4, two=2)
                 for j in range(4)]
        s2d_sb = [[main.tile([128, 512], bf16, tag=f"s2d{par}{g}",
                             name=f"s2d{par}{g}") for g in range(4)]
                  for par in range(2)]
        # per-parity output arenas: one merged 4-block DMA per parity
        ostp = [main.tile([128, 4 * 512], bf16, tag=f"ostp{par}",
                          name=f"ostp{par}") for par in range(2)]

        with tc.tile_pool(name="psTail", bufs=1, space="PSUM") as psT:
            held = {}

            def emit_proj_pre(par, g):
                acc = psT.tile([128, 512], f32, tag="pj", bufs=8)
                for j in range(3):
                    nc.tensor.matmul(acc[:], rtb_v[j][:, g, par, :],
                                     wp_c[j][:, :], start=(j == 0), stop=False)
                held[(par, g)] = acc

            def emit_proj_fin(par, g, eng):
                acc = held.pop((par, g))
                nc.tensor.matmul(acc[:], rtb_v[3][:, g, par, :],
                                 wp_c[3][:, :], start=False, stop=True)
                eng(s2d_sb[par][g][:], acc[:])

            def emit_conv_s(par):
                for oc in range(4):
                    acc = psT.tile([128, 512], f32, tag="pj", bufs=8)
                    for g in range(4):
                        nc.tensor.matmul(acc[:],
                                         wcs_c[g][:, 128 * oc:128 * (oc + 1)],
                                         s2d_sb[par][g][:],
                                         start=(g == 0), stop=(g == 3))
                    eng = (nc.vector.tensor_copy if oc % 2 == 0
                           else nc.scalar.copy)
                    eng(ostp[par][:, 512 * oc:512 * (oc + 1)], acc[:])
                out3 = out_p.rearrange("(oc p) s -> p oc s", oc=4)
                src3 = ostp[par][:].rearrange("p (oc s) -> p oc s", oc=4)
                nc.sync.dma_start(out3[:, :, 512 * par:512 * (par + 1)],
                                  src3[:, :, :])

            # pre-start j0..2 (needs only rtb0-2; banks free as the last
            # head's exps drain the sc pool)
            for g in range(4):
                emit_proj_pre(0, g)
            for g in range(4):
                emit_proj_pre(1, g)
            for g in range(4):
                emit_proj_fin(0, g, nc.vector.tensor_copy if g % 2 == 0
                              else nc.scalar.copy)
            for g in range(4):
                emit_proj_fin(1, g, nc.vector.tensor_copy if g % 2 == 0
                              else nc.scalar.copy)
            emit_conv_s(0)
            emit_conv_s(1)

    nc.compile()
    _CACHE["nc"] = nc
    return nc


def _shard_inputs(content_feat, components, pos_emb, Wq, Wkv, Wproj, bproj,
                  Wconv, bconv):
    import ml_dtypes

    bf = ml_dtypes.bfloat16
    f = np.float32
    pos2 = np.asarray(pos_emb, dtype=f).reshape(S, C)
    wq16 = np.asarray(Wq, dtype=f).astype(bf)
    wkv16 = np.asarray(Wkv, dtype=f).astype(bf)
    wp16 = np.asarray(Wproj, dtype=f).astype(bf)
    wcT = np.ascontiguousarray(np.asarray(Wconv, dtype=f).T)
    wcs16 = np.ascontiguousarray(wcT[:C]).astype(bf)
    in_maps = []
    for core in range(N_CORES):
        b, n = core // 4, core % 4
        ctokT = np.ascontiguousarray(
            (np.asarray(content_feat[b], dtype=f).reshape(S, C) + pos2).T)
        compT = np.ascontiguousarray(
            (np.asarray(components[n, b], dtype=f).reshape(S, C) + pos2).T)
        in_maps.append({
            "comp16": compT.astype(bf),
            "ctok16": ctokT.astype(bf),
            "cf16": np.ascontiguousarray(
                np.asarray(content_feat[b], dtype=f).reshape(C, S)).astype(bf),
            "wq16": wq16,
            "wkv16": wkv16,
            "wp16": wp16,
            "wcs16": wcs16,
            "wcc16": np.ascontiguousarray(
                wcT[C:, 128 * n:128 * (n + 1)]).astype(bf),
            "gate": np.full((128, 1), 1.0 if n == 0 else 0.0, dtype=f),
        })
    return in_maps


def _run(trace=False, **inputs):
    from concourse.bass_utils import run_bass_kernel_spmd

    nc = _build()
    in_maps = _shard_inputs(**inputs)
    res = run_bass_kernel_spmd(nc, in_maps, list(range(N_CORES)), trace=trace)
    outs = [np.asarray(res.results[i]["out_p"], dtype=np.float64)
            for i in range(N_CORES)]
    out = np.stack([outs[0] + outs[1] + outs[2] + outs[3],
                    outs[4] + outs[5] + outs[6] + outs[7]], axis=0)
    for core in range(N_CORES):
        b, n = core // 4, core % 4
        out[b, 128 * n:128 * (n + 1), :] += np.asarray(
            res.results[core]["out_cf"], dtype=np.float64)
    # host-side affine constants: out += bconv[o] + ws[o]*bproj[i%512]
    # with ws[o] = sum_c Wconv[o, c<C]  (bproj enters via the conv s-half)
    Wconv = np.asarray(inputs["Wconv"], dtype=np.float64)
    bproj = np.asarray(inputs["bproj"], dtype=np.float64)
    bconv = np.asarray(inputs["bconv"], dtype=np.float64)
    ws = Wconv[:, :C].sum(axis=1)
    bias_img = bconv[:, None] + np.outer(ws, np.concatenate([bproj, bproj]))
    out += bias_img[None, :, :]
    return out.reshape(B, C, H, W).astype(np.float32), res


def kernel(**inputs):
    out, _ = _run(trace=False, **inputs)
    return out


# revision 17
# speedup vs baseline: 1.1291x; 1.0533x over previous
"""Trainium2 Bass kernel for nn_Attention_54391465836966.

Math (per batch b, component n; one (b,n) pair per core, host sums over n):
  ctok = content_feat[b].raw_reshape(S,C) + pos          # [1024, 512]
  comp_tok = components[n,b].raw_reshape(S,C) + pos
  q = ctok @ Wq ; k,v = comp_tok @ Wkv (split)
  per head h: P = exp(scale q_h k_h^T); o_h = (P @ v_h) / rowsum(P)
  s_in = concat_h(o_h) + gate*ctok                        # gate = [n==0]
  s2d  = raw_reshape(s_in @ Wproj)                        # bias via host
  out  = Wconv^T[:C] @ s2d + Wconv^T[C:] @ cf             # + bias via host

Design notes (cost-model-driven):
- The P@V (o) matmuls run fp8 DoubleRow (2 k-tiles per instruction at 0.5
  cycles/row = 4x the bf16 rate): pt = exp output in e5m2 (its 22-efold
  dynamic range covers the unnormalized exp spread -- score sigma ~2.0 ->
  +-11 efolds; e4m3's 11.7 efolds would overflow), v drained to e4m3.
  Attention-output noise from fp8 is diluted by the ctok residual and
  P-quantization partially cancels against Z (measured 1.4e-2 total).
- The q/k path stays bf16 end-to-end: fp8 anywhere before the exp adds
  ~0.17 absolute score noise -> 17% P-reweighting with only ~19 effective
  keys after softmax (score sigma 2.0) -> 3-4e-2 output error (measured).
- exp writes pt e5m2 directly with a constant -4 bias (normalization
  cancels it exactly; caps pt at exp(score-4) with overflow only past
  7.5 sigma scores, flush below -7.1 negligible).
- Z rides row 0 of the o PSUM tile (ones column first in v): the custom-DVE
  reciprocal reads partition 0 of PSUM directly (hw quirk: it ignores the
  AP's partition base), skipping the per-head z-copy entirely.  Drain +
  normalize fuse into one scalar_tensor_tensor from PSUM (v data sits at
  e=64:128 so the o data rows land at the 32-aligned partition base 64).
- The projection uses stride-2 column slices of s_in^T as the stationary
  operand, which makes its PSUM output land directly in the s->s2d
  raw-reshape layout: no PE transposes anywhere in the kernel.
- bproj/bconv are affine constants independent of the data path; their
  contribution is bconv[o] + (sum_c Wconv[o,c<C]) * bproj[i%512], a rank-1
  image the host adds during unshard.
- PSUM: score tiles tag "sc" [128,1024] bufs=3 (6 banks) shared by the
  QKV-projection accumulations (pre-phase + interleaved jobs borrow
  rotation slots); o tag [128,1024] bufs=1 (2 banks).  GPSIMD cannot touch
  PSUM, so all PSUM drains go to DVE (ACT helps only post-exp in the tail).
- ACT runs exps only (64 x [128,1024] ~= 1.04us each); QKV/conv_cf jobs are
  split into ~2048-cycle half-jobs and interleaved between score matmuls so
  the exp stream never starves.  o matmuls for head h trail into head h+1's
  first slots; the recip->broadcast->normalize chain then frees the single
  o buffer before head h+1 needs it.
"""
import sys

sys.path.insert(0, "/opt/trn_rl_repo")

import numpy as np

N_CORES = 8
B, C, H, W = 2, 512, 32, 32
S = H * W  # 1024
NH, HD = 8, 64
SCALE = HD ** -0.5
EXP_BIAS = -4.0

_CACHE = {}


def _build():
    if "nc" in _CACHE:
        return _CACHE["nc"]
    from contextlib import ExitStack

    import concourse.bacc as bacc
    import concourse.mybir as mybir
    import concourse.tile as tile

    f32 = mybir.dt.float32
    bf16 = mybir.dt.bfloat16
    fp8 = mybir.dt.float8e4
    fp8e5 = mybir.dt.float8e5
    EXP = mybir.ActivationFunctionType.Exp
    MULT = mybir.AluOpType.mult
    ADD = mybir.AluOpType.add
    BYPASS = mybir.AluOpType.bypass
    DR = mybir.MatmulPerfMode.DoubleRow

    nc = bacc.Bacc("TRN2", target_bir_lowering=False, debug=False,
                   num_devices=N_CORES)

    din = lambda n, s, dt: nc.dram_tensor(n, s, dt, kind="ExternalInput").ap()
    comp16d = din("comp16", [C, S], bf16)   # (comp_tok + pos).T
    ctok16d = din("ctok16", [C, S], bf16)   # (content_tok + pos).T
    cf16d = din("cf16", [C, S], bf16)       # content_feat[b] raw [C,S]
    wq16d = din("wq16", [C, C], bf16)       # Wq
    wkv16d = din("wkv16", [C, 2 * C], bf16)  # Wkv (cols 0:C K, C:2C V)
    wp16d = din("wp16", [C, C], bf16)       # Wproj
    wcs16d = din("wcs16", [C, C], bf16)     # Wconv.T rows :C (s half)
    wcc16d = din("wcc16", [C, 128], bf16)   # Wconv.T[C:, 128n:128(n+1)]
    gated = din("gate", [128, 1], f32)      # 1.0 on n==0 cores else 0.0
    out_p = nc.dram_tensor("out_p", [C, S], bf16, kind="ExternalOutput").ap()
    out_cf = nc.dram_tensor("out_cf", [128, S], bf16,
                            kind="ExternalOutput").ap()

    with tile.TileContext(nc) as tc, ExitStack() as ctx:
        main = ctx.enter_context(tc.tile_pool(name="main", bufs=1))

        g_sb = main.tile([128, 1], f32, tag="g")
        ebias = main.tile([128, 1], f32, tag="eb")
        nc.gpsimd.memset(ebias[:], EXP_BIAS)

        # ---- persistent SBUF tiles (one merged DMA per DRAM tensor) ----
        comp16 = main.tile([128, 4 * S], bf16, tag="cm16", name="cm16")
        ctok16 = main.tile([128, 4 * S], bf16, tag="ct16", name="ct16")
        cf16 = main.tile([128, 4 * S], bf16, tag="cf16", name="cf16")
        wq16 = main.tile([128, 4 * C], bf16, tag="wq16", name="wq16")
        wkv16 = main.tile([128, 4 * 2 * C], bf16, tag="wkv16", name="wkv16")
        wp16 = main.tile([128, 4 * C], bf16, tag="wp16", name="wp16")
        wcs16 = main.tile([128, 4 * C], bf16, tag="wcs16", name="wcs16")
        wcc16 = main.tile([128, 4 * 128], bf16, tag="wcc16", name="wcc16")

        comp_c = [comp16[:, S * k:S * (k + 1)] for k in range(4)]
        ctok_c = [ctok16[:, S * k:S * (k + 1)] for k in range(4)]
        cf_c = [cf16[:, S * k:S * (k + 1)] for k in range(4)]
        wq_c = [wq16[:, C * k:C * (k + 1)] for k in range(4)]
        wkv_c = [wkv16[:, 2 * C * k:2 * C * (k + 1)] for k in range(4)]
        wp_c = [wp16[:, C * k:C * (k + 1)] for k in range(4)]
        wcs_c = [wcs16[:, C * k:C * (k + 1)] for k in range(4)]
        wcc_c = [wcc16[:, 128 * k:128 * (k + 1)] for k in range(4)]

        kT16 = [main.tile([128, S], bf16, tag=f"kt{j}", name=f"kt{j}")
                for j in range(4)]
        qT16 = [main.tile([128, S], bf16, tag=f"qt{j}", name=f"qt{j}")
                for j in range(4)]
        # v pair tiles: [128 keys, (t2=2, h=8, e=128)] fp8e4; e=0 is the 1.0
        # ones column (Z lands on o row 0), e=64:128 the v data, 1:64 zeros.
        vp = [main.tile([128, 2 * 8 * 128], fp8, tag=f"vp{p}", name=f"vp{p}")
              for p in range(4)]
        rtb = [main.tile([128, S], bf16, tag=f"rt{j}", name=f"rt{j}")
               for j in range(4)]
        outcf_sb = main.tile([128, S], bf16, tag="ocf", name="ocf")

        # ---- DMA emission order: attention-critical first ----
        def dma_merged(dst_tile, src_ap, k, lo=0, hi=None):
            hi = k if hi is None else hi
            src3 = src_ap.rearrange("(k p) s -> p k s", k=k)
            dst3 = dst_tile[:].rearrange("p (k s) -> p k s", k=k)
            nc.sync.dma_start(dst3[:, lo:hi, :], src3[:, lo:hi, :])

        def dma_cols(dst_tile, src_ap, k, c0, c1):
            # column-group slice across all k row-chunks in one strided DMA
            src3 = src_ap.rearrange("(k p) s -> p k s", k=k)
            dst3 = dst_tile[:].rearrange("p (k s) -> p k s", k=k)
            nc.sync.dma_start(dst3[:, :, c0:c1], src3[:, :, c0:c1])

        # critical path for the first score matmul: comp + ctok + the j0
        # column groups of Wk/Wq (~2.25MB); everything else after
        dma_merged(comp16, comp16d, 4, 0, 2)
        dma_cols(wkv16, wkv16d, 4, 0, 128)        # Wk j0 cols
        dma_merged(comp16, comp16d, 4, 2, 4)
        dma_merged(ctok16, ctok16d, 4, 0, 2)
        dma_cols(wq16, wq16d, 4, 0, 128)          # Wq j0 cols
        dma_merged(ctok16, ctok16d, 4, 2, 4)
        nc.sync.dma_start(g_sb[:], gated[:])
        dma_cols(wkv16, wkv16d, 4, 512, 1024)     # V half (v jobs, early)
        dma_cols(wkv16, wkv16d, 4, 128, 512)      # Wk j1-3
        dma_cols(wq16, wq16d, 4, 128, 512)        # Wq j1-3
        dma_merged(cf16, cf16d, 4)
        dma_merged(wcc16, wcc16d, 4)
        dma_merged(wp16, wp16d, 4)
        dma_merged(wcs16, wcs16d, 4)

        # ones + zero-pad columns of the v tiles (SBUF memset = Pool)
        for p in range(4):
            vv = vp[p][:].rearrange("p (t h e) -> p t h e", t=2, h=8)
            nc.gpsimd.memset(vv[:, :, :, 0:1], 1.0)
            nc.gpsimd.memset(vv[:, :, :, 1:64], 0.0)

        warm_src = main.tile([128, 128], bf16, tag="warm", name="warm")
        nc.gpsimd.memset(warm_src[:], 0.25)
        # preload the Exp activation table (1.28us) off the critical stream
        dummy_pt = main.tile([1, 8], fp8e5, tag="dpt", name="dpt")
        nc.scalar.activation(dummy_pt[0:1, :], warm_src[0:1, 0:8], EXP,
                             bias=ebias[0:1, 0:1], scale=SCALE)

        with tc.tile_pool(name="psMain", bufs=1, space="PSUM") as ps:
            # p-state spin-up gated on the first comp chunk so the ramp is
            # still warm when the first kq matmuls run (dep-free warms would
            # finish during the DMA wait and let the clock reset)
            for _ in range(28):
                wtp = ps.tile([128, 1024], f32, tag="sc", bufs=3)
                nc.tensor.matmul(wtp[:, 0:128], warm_src[:],
                                 comp16[:, 0:128], start=True, stop=True)

            # ---- QKV projection half-jobs (bf16, ~2048 PE cycles each) ----
            kq_acc = {}

            def emit_kq_half(dst, w_c_, x_c_, j, t):
                if (id(dst), j) not in kq_acc:
                    kq_acc[(id(dst), j)] = ps.tile(
                        [128, 1024], f32, tag="sc", bufs=3, name=f"kqa{j}")
                acc = kq_acc[(id(dst), j)]
                for k in range(4):
                    nc.tensor.matmul(
                        acc[:, 512 * t:512 * (t + 1)],
                        w_c_[k][:, 128 * j:128 * (j + 1)],
                        x_c_[k][:, 512 * t:512 * (t + 1)],
                        start=(k == 0), stop=(k == 3))
                nc.vector.tensor_copy(dst[:, 512 * t:512 * (t + 1)],
                                      acc[:, 512 * t:512 * (t + 1)])
                if t == 1:
                    del kq_acc[(id(dst), j)]

            v_acc = {}

            def emit_v_half(p, t2):
                # token block 2p+t2; drain (strided, ->fp8e4) at t2==1
                if p not in v_acc:
                    v_acc[p] = ps.tile([128, 1024], f32, tag="sc", bufs=3,
                                       name=f"va{p}")
                acc = v_acc[p]
                tb = 2 * p + t2
                for k in range(4):
                    nc.tensor.matmul(
                        acc[:, 512 * t2:512 * (t2 + 1)],
                        comp_c[k][:, 128 * tb:128 * (tb + 1)],
                        wkv_c[k][:, C:2 * C],
                        start=(k == 0), stop=(k == 3))
                if t2 == 1:
                    del v_acc[p]
                    dst = vp[p][:].rearrange("p (t h e) -> p t h e", t=2, h=8)
                    src = acc[:].rearrange("p (t h d) -> p t h d", t=2, h=8)
                    nc.vector.tensor_copy(dst[:, :, :, 64:128],
                                          src[:, :, :, :])

            def emit_conv_cf(pc):
                acc = ps.tile([128, 1024], f32, tag="sc", bufs=3)
                for g in range(4):
                    nc.tensor.matmul(acc[:, 0:512], wcc_c[g][:, :],
                                     cf_c[g][:, 512 * pc:512 * (pc + 1)],
                                     start=(g == 0), stop=(g == 3))
                nc.vector.tensor_copy(
                    outcf_sb[:, 512 * pc:512 * (pc + 1)], acc[:, 0:512])
                nc.sync.dma_start(
                    out_cf[:, 512 * pc:512 * (pc + 1)],
                    outcf_sb[:, 512 * pc:512 * (pc + 1)])

            # pre-phase: just enough for sc(0,0): kT0 keys 0:512, qT0 full
            emit_kq_half(kT16[0], wkv_c, comp_c, 0, 0)
            emit_kq_half(qT16[0], wq_c, ctok_c, 0, 0)
            emit_kq_half(qT16[0], wq_c, ctok_c, 0, 1)

            # interleaved half-jobs, keyed by the (head, kt) slot AFTER whose
            # score-matmuls they are emitted.  v pair p is needed by the o
            # matmul at slot (h,6)/(h,7)/(h+1,0)/(h+1,1); kT/qT j by (2j,0).
            ilv = {
                (0, 0): lambda: emit_kq_half(kT16[0], wkv_c, comp_c, 0, 1),
                (0, 1): lambda: emit_v_half(0, 0),
                (0, 2): lambda: emit_v_half(0, 1),
                (0, 3): lambda: emit_v_half(1, 0),
                (0, 4): lambda: emit_v_half(1, 1),
                (0, 5): lambda: emit_v_half(2, 0),
                (0, 6): lambda: emit_v_half(2, 1),
                (0, 7): lambda: emit_v_half(3, 0),
                (1, 0): lambda: emit_v_half(3, 1),
                (1, 2): lambda: emit_kq_half(kT16[1], wkv_c, comp_c, 1, 0),
                (1, 3): lambda: emit_kq_half(kT16[1], wkv_c, comp_c, 1, 1),
                (1, 4): lambda: emit_kq_half(qT16[1], wq_c, ctok_c, 1, 0),
                (1, 5): lambda: emit_kq_half(qT16[1], wq_c, ctok_c, 1, 1),
                (2, 2): lambda: emit_kq_half(kT16[2], wkv_c, comp_c, 2, 0),
                (2, 3): lambda: emit_kq_half(kT16[2], wkv_c, comp_c, 2, 1),
                (2, 4): lambda: emit_kq_half(qT16[2], wq_c, ctok_c, 2, 0),
                (2, 5): lambda: emit_kq_half(qT16[2], wq_c, ctok_c, 2, 1),
                (3, 2): lambda: emit_kq_half(kT16[3], wkv_c, comp_c, 3, 0),
                (3, 3): lambda: emit_kq_half(kT16[3], wkv_c, comp_c, 3, 1),
                (3, 4): lambda: emit_kq_half(qT16[3], wq_c, ctok_c, 3, 0),
                (3, 5): lambda: emit_kq_half(qT16[3], wq_c, ctok_c, 3, 1),
                (7, 2): lambda: emit_conv_cf(0),
                (7, 4): lambda: emit_conv_cf(1),
            }

            # ---- attention ----
            pt_pool = {}      # (h, p) -> pt pair tile [128, 2048] e5m2
            o_tiles = {}      # h -> o psum tile

            def emit_sc(h, kt):
                jq, row = h // 2, 64 * (h % 2)
                sc = ps.tile([128, S], f32, tag="sc", bufs=3)
                for qc in range(2):
                    nc.tensor.matmul(
                        sc[:, 512 * qc:512 * (qc + 1)],
                        kT16[jq][row:row + 64, 128 * kt:128 * (kt + 1)],
                        qT16[jq][row:row + 64, 512 * qc:512 * (qc + 1)],
                        start=True, stop=True)
                if kt % 2 == 0:
                    pt_pool[(h, kt // 2)] = main.tile(
                        [128, 2048], fp8e5, tag="pt", bufs=6,
                        name=f"pt{h}_{kt // 2}")
                pt = pt_pool[(h, kt // 2)]
                nc.scalar.activation(pt[:, 1024 * (kt % 2):1024 * (kt % 2 + 1)],
                                     sc[:], EXP, bias=ebias[:, 0:1],
                                     scale=SCALE)

            def emit_o(h, p):
                # o[0,:] = Z, o[64:128,:] = P@v_h  (DoubleRow over kt pair)
                if h not in o_tiles:
                    o_tiles[h] = ps.tile([128, S], f32, tag="o", bufs=1,
                                         name=f"o{h}")
                o_ps = o_tiles[h]
                vv = vp[p][:].rearrange("p (t h e) -> p t h e", t=2, h=8)
                ptv = pt_pool.pop((h, p))[:].rearrange("p (t q) -> p t q", t=2)
                for qc in range(2):
                    nc.tensor.matmul(
                        o_ps[:, 512 * qc:512 * (qc + 1)],
                        vv[:, :, h:h + 1, :],
                        ptv[:, :, 512 * qc:512 * (qc + 1)],
                        start=(p == 0), stop=(p == 3), perf_mode=DR)

            def emit_norm(h, split=False):
                jq, row = h // 2, 64 * (h % 2)
                o_ps = o_tiles.pop(h)
                if split:
                    # qc-halved chain: shorter serial latency on the tail
                    for qc in range(2):
                        zi = main.tile([1, 512], f32, tag="zis", bufs=2,
                                       name=f"zis{h}{qc}")
                        zbc = main.tile([64, 512], f32, tag="zbs", bufs=2,
                                        name=f"zbs{h}{qc}")
                        nc.vector.reciprocal_approx_fast(
                            zi[0:1, :], o_ps[0:1, 512 * qc:512 * (qc + 1)])
                        nc.gpsimd.partition_broadcast(zbc[0:64, :],
                                                      zi[0:1, :])
                        nc.vector.scalar_tensor_tensor(
                            rtb[jq][row:row + 64, 512 * qc:512 * (qc + 1)],
                            o_ps[64:128, 512 * qc:512 * (qc + 1)], 1.0,
                            zbc[0:64, :], BYPASS, MULT)
                else:
                    zi = main.tile([1, S], f32, tag="zi", bufs=2,
                                   name=f"zi{h}")
                    zbc = main.tile([64, S], f32, tag="zb", bufs=2,
                                    name=f"zb{h}")
                    nc.vector.reciprocal_approx_fast(zi[0:1, :], o_ps[0:1, :])
                    nc.gpsimd.partition_broadcast(zbc[0:64, :], zi[0:1, :])
                    nc.vector.scalar_tensor_tensor(
                        rtb[jq][row:row + 64, :], o_ps[64:128, :], 1.0,
                        zbc[0:64, :], BYPASS, MULT)
                if h % 2 == 1:  # pair complete: s_in^T = rtb + gate*ctokT
                    nc.vector.scalar_tensor_tensor(
                        rtb[jq][:], ctok_c[jq][:], g_sb[:, 0:1], rtb[jq][:],
                        MULT, ADD)

            # flat emission; o(h) trails into head h+1 per the o-bank cycle
            for h in range(NH):
                for kt in range(8):
                    emit_sc(h, kt)
                    if h > 0:
                        if kt == 0:
                            emit_o(h - 1, 2)
                        elif kt == 1:
                            emit_o(h - 1, 3)
                        elif kt == 2:
                            emit_norm(h - 1)
                    if (h, kt) in ilv:
                        ilv[(h, kt)]()
                    if kt == 6:
                        emit_o(h, 0)
                    elif kt == 7:
                        emit_o(h, 1)
            # ---- tail (same pool: proj/conv accumulators ride the
            # sc/o tags as [128,512] halves; proj j0-2 partials fill the
            # end of the exp stream before the last o matmuls) ----
            rtb_v = [rtb[j][:].rearrange("p (g i two) -> p g two i",
                                         g=4, two=2) for j in range(4)]
            s2d_sb = [[main.tile([128, 512], bf16, tag=f"s2d{par}{g}",
                                 name=f"s2d{par}{g}") for g in range(4)]
                      for par in range(2)]
            ostp = [main.tile([128, 4 * 512], bf16, tag=f"ostp{par}",
                              name=f"ostp{par}") for par in range(2)]
            pj = {}

            def emit_pre2(pairs, tag):
                tl_ = ps.tile([128, 1024], f32, tag=tag,
                              bufs=3 if tag == "sc" else 1,
                              name=f"pj{pairs[0][0]}{pairs[0][1]}")
                for i, (par, g) in enumerate(pairs):
                    acc = tl_[:, 512 * i:512 * (i + 1)]
                    pj[(par, g)] = acc
                    for j in range(3):
                        nc.tensor.matmul(acc, rtb_v[j][:, g, par, :],
                                         wp_c[j][:, :], start=(j == 0),
                                         stop=False)

            def emit_fin(par, g, eng):
                acc = pj.pop((par, g))
                nc.tensor.matmul(acc, rtb_v[3][:, g, par, :], wp_c[3][:, :],
                                 start=False, stop=True)
                eng(s2d_sb[par][g][:], acc)

            def emit_conv_s(par):
                for ocp in range(2):
                    tl_ = ps.tile([128, 1024], f32, tag="sc", bufs=3,
                                  name=f"cv{par}{ocp}")
                    for i in range(2):
                        oc = 2 * ocp + i
                        acc = tl_[:, 512 * i:512 * (i + 1)]
                        for g in range(4):
                            nc.tensor.matmul(
                                acc, wcs_c[g][:, 128 * oc:128 * (oc + 1)],
                                s2d_sb[par][g][:],
                                start=(g == 0), stop=(g == 3))
                        eng = (nc.vector.tensor_copy if oc % 2 == 0
                               else nc.scalar.copy)
                        eng(ostp[par][:, 512 * oc:512 * (oc + 1)], acc)
                out3 = out_p.rearrange("(oc p) s -> p oc s", oc=4)
                src3 = ostp[par][:].rearrange("p (oc s) -> p oc s", oc=4)
                nc.sync.dma_start(out3[:, :, 512 * par:512 * (par + 1)],
                                  src3[:, :, :])

            emit_pre2([(0, 0), (0, 1)], "sc")
            emit_pre2([(0, 2), (0, 3)], "sc")
            emit_o(NH - 1, 2)
            emit_pre2([(1, 0), (1, 1)], "sc")
            emit_o(NH - 1, 3)
            emit_norm(NH - 1, split=True)
            emit_pre2([(1, 2), (1, 3)], "o")
            for g in range(4):
                emit_fin(0, g, nc.vector.tensor_copy if g % 2 == 0
                         else nc.scalar.copy)
            for g in range(4):
                emit_fin(1, g, nc.vector.tensor_copy if g % 2 == 0
                         else nc.scalar.copy)
            emit_conv_s(0)
            emit_conv_s(1)

    nc.compile()
    _CACHE["nc"] = nc
    return nc


def _shard_inputs(content_feat, components, pos_emb, Wq, Wkv, Wproj, bproj,
                  Wconv, bconv):
    import ml_dtypes

    bf = ml_dtypes.bfloat16
    f = np.float32
    pos2 = np.asarray(pos_emb, dtype=f).reshape(S, C)
    wq16 = np.asarray(Wq, dtype=f).astype(bf)
    wkv16 = np.asarray(Wkv, dtype=f).astype(bf)
    wp16 = np.asarray(Wproj, dtype=f).astype(bf)
    wcT = np.ascontiguousarray(np.asarray(Wconv, dtype=f).T)
    wcs16 = np.ascontiguousarray(wcT[:C]).astype(bf)
    in_maps = []
    for core in range(N_CORES):
        b, n = core // 4, core % 4
        ctokT = np.ascontiguousarray(
            (np.asarray(content_feat[b], dtype=f).reshape(S, C) + pos2).T)
        compT = np.ascontiguousarray(
            (np.asarray(components[n, b], dtype=f).reshape(S, C) + pos2).T)
        in_maps.append({
            "comp16": compT.astype(bf),
            "ctok16": ctokT.astype(bf),
            "cf16": np.ascontiguousarray(
                np.asarray(content_feat[b], dtype=f).reshape(C, S)).astype(bf),
            "wq16": wq16,
            "wkv16": wkv16,
            "wp16": wp16,
            "wcs16": wcs16,
            "wcc16": np.ascontiguousarray(
                wcT[C:, 128 * n:128 * (n + 1)]).astype(bf),
            "gate": np.full((128, 1), 1.0 if n == 0 else 0.0, dtype=f),
        })
    return in_maps


def _run(trace=False, **inputs):
    from concourse.bass_utils import run_bass_kernel_spmd

    nc = _build()
    in_maps = _shard_inputs(**inputs)
    res = run_bass_kernel_spmd(nc, in_maps, list(range(N_CORES)), trace=trace)
    outs = [np.asarray(res.results[i]["out_p"], dtype=np.float64)
            for i in range(N_CORES)]
    out = np.stack([outs[0] + outs[1] + outs[2] + outs[3],
                    outs[4] + outs[5] + outs[6] + outs[7]], axis=0)
    for core in range(N_CORES):
        b, n = core // 4, core % 4
        out[b, 128 * n:128 * (n + 1), :] += np.asarray(
            res.results[core]["out_cf"], dtype=np.float64)
    # host-side affine constants: out += bconv[o] + ws[o]*bproj[i%512]
    # with ws[o] = sum_c Wconv[o, c<C]  (bproj enters via the conv s-half)
    Wconv = np.asarray(inputs["Wconv"], dtype=np.float64)
    bproj = np.asarray(inputs["bproj"], dtype=np.float64)
    bconv = np.asarray(inputs["bconv"], dtype=np.float64)
    ws = Wconv[:, :C].sum(axis=1)
    bias_img = bconv[:, None] + np.outer(ws, np.concatenate([bproj, bproj]))
    out += bias_img[None, :, :]
    return out.reshape(B, C, H, W).astype(np.float32), res


def kernel(**inputs):
    out, _ = _run(trace=False, **inputs)
    return out


# revision 19
# speedup vs baseline: 1.2100x; 1.0717x over previous
"""Trainium2 Bass kernel for nn_Attention_54391465836966.

Math (per batch b, component n; one (b,n) pair per core, host sums over n):
  ctok = content_feat[b].raw_reshape(S,C) + pos          # [1024, 512]
  comp_tok = components[n,b].raw_reshape(S,C) + pos
  q = ctok @ Wq ; k,v = comp_tok @ Wkv (split)
  per head h: P = exp(scale q_h k_h^T); o_h = (P @ v_h) / rowsum(P)
  s_in = concat_h(o_h) + gate*ctok                        # gate = [n==0]
  s2d  = raw_reshape(s_in @ Wproj)                        # bias via host
  out  = Wconv^T[:C] @ s2d + Wconv^T[C:] @ cf             # + bias via host

Design notes (cost-model-driven):
- The P@V (o) matmuls run fp8 DoubleRow (2 k-tiles per instruction at 0.5
  cycles/row = 4x the bf16 rate): pt = exp output in e5m2 (its 22-efold
  dynamic range covers the unnormalized exp spread -- score sigma ~2.0 ->
  +-11 efolds; e4m3's 11.7 efolds would overflow), v drained to e4m3.
  Attention-output noise from fp8 is diluted by the ctok residual and
  P-quantization partially cancels against Z (measured 1.4e-2 total).
- The q/k path stays bf16 end-to-end: fp8 anywhere before the exp adds
  ~0.17 absolute score noise -> 17% P-reweighting with only ~19 effective
  keys after softmax (score sigma 2.0) -> 3-4e-2 output error (measured).
- exp writes pt e5m2 directly with a constant -4 bias (normalization
  cancels it exactly; caps pt at exp(score-4) with overflow only past
  7.5 sigma scores, flush below -7.1 negligible).
- Z rides row 0 of the o PSUM tile (ones column first in v): the custom-DVE
  reciprocal reads partition 0 of PSUM directly (hw quirk: it ignores the
  AP's partition base), skipping the per-head z-copy entirely.  Drain +
  normalize fuse into one scalar_tensor_tensor from PSUM (v data sits at
  e=64:128 so the o data rows land at the 32-aligned partition base 64).
- The projection uses stride-2 column slices of s_in^T as the stationary
  operand, which makes its PSUM output land directly in the s->s2d
  raw-reshape layout: no PE transposes anywhere in the kernel.
- bproj/bconv are affine constants independent of the data path; their
  contribution is bconv[o] + (sum_c Wconv[o,c<C]) * bproj[i%512], a rank-1
  image the host adds during unshard.
- PSUM: score tiles tag "sc" [128,1024] bufs=3 (6 banks) shared by the
  QKV-projection accumulations (pre-phase + interleaved jobs borrow
  rotation slots); o tag [128,1024] bufs=1 (2 banks).  GPSIMD cannot touch
  PSUM, so all PSUM drains go to DVE (ACT helps only post-exp in the tail).
- ACT runs exps only (64 x [128,1024] ~= 1.04us each); QKV/conv_cf jobs are
  split into ~2048-cycle half-jobs and interleaved between score matmuls so
  the exp stream never starves.  o matmuls for head h trail into head h+1's
  first slots; the recip->broadcast->normalize chain then frees the single
  o buffer before head h+1 needs it.
"""
import sys

sys.path.insert(0, "/opt/trn_rl_repo")

import numpy as np

N_CORES = 8
B, C, H, W = 2, 512, 32, 32
S = H * W  # 1024
NH, HD = 8, 64
SCALE = HD ** -0.5
EXP_BIAS = -4.0

_CACHE = {}


def _build():
    if "nc" in _CACHE:
        return _CACHE["nc"]
    from contextlib import ExitStack

    import concourse.bacc as bacc
    import concourse.mybir as mybir
    import concourse.tile as tile

    f32 = mybir.dt.float32
    bf16 = mybir.dt.bfloat16
    fp8 = mybir.dt.float8e4
    fp8e5 = mybir.dt.float8e5
    EXP = mybir.ActivationFunctionType.Exp
    MULT = mybir.AluOpType.mult
    ADD = mybir.AluOpType.add
    BYPASS = mybir.AluOpType.bypass
    DR = mybir.MatmulPerfMode.DoubleRow

    nc = bacc.Bacc("TRN2", target_bir_lowering=False, debug=False,
                   num_devices=N_CORES)

    din = lambda n, s, dt: nc.dram_tensor(n, s, dt, kind="ExternalInput").ap()
    comp16d = din("comp16", [C, S], bf16)   # (comp_tok + pos).T
    ctok16d = din("ctok16", [C, S], bf16)   # (content_tok + pos).T
    cf16d = din("cf16", [C, S], bf16)       # content_feat[b] raw [C,S]
    wq16d = din("wq16", [C, C], bf16)       # Wq
    wkv16d = din("wkv16", [C, 2 * C], bf16)  # Wkv (cols 0:C K, C:2C V)
    wp16d = din("wp16", [C, C], bf16)       # Wproj
    wcs16d = din("wcs16", [C, C], bf16)     # Wconv.T rows :C (s half)
    wcc16d = din("wcc16", [C, 128], bf16)   # Wconv.T[C:, 128n:128(n+1)]
    gated = din("gate", [128, 1], f32)      # 1.0 on n==0 cores else 0.0
    out_p = nc.dram_tensor("out_p", [C, S], bf16, kind="ExternalOutput").ap()
    out_cf = nc.dram_tensor("out_cf", [128, S], bf16,
                            kind="ExternalOutput").ap()

    with tile.TileContext(nc) as tc, ExitStack() as ctx:
        main = ctx.enter_context(tc.tile_pool(name="main", bufs=1))

        g_sb = main.tile([128, 1], f32, tag="g")
        ebias = main.tile([128, 1], f32, tag="eb")
        nc.gpsimd.memset(ebias[:], EXP_BIAS)

        # ---- persistent SBUF tiles (one merged DMA per DRAM tensor) ----
        comp16 = main.tile([128, 4 * S], bf16, tag="cm16", name="cm16")
        ctok16 = main.tile([128, 4 * S], bf16, tag="ct16", name="ct16")
        cf16 = main.tile([128, 4 * S], bf16, tag="cf16", name="cf16")
        wq16 = main.tile([128, 4 * C], bf16, tag="wq16", name="wq16")
        wkv16 = main.tile([128, 4 * 2 * C], bf16, tag="wkv16", name="wkv16")
        wp16 = main.tile([128, 4 * C], bf16, tag="wp16", name="wp16")
        wcs16 = main.tile([128, 4 * C], bf16, tag="wcs16", name="wcs16")
        wcc16 = main.tile([128, 4 * 128], bf16, tag="wcc16", name="wcc16")

        comp_c = [comp16[:, S * k:S * (k + 1)] for k in range(4)]
        ctok_c = [ctok16[:, S * k:S * (k + 1)] for k in range(4)]
        cf_c = [cf16[:, S * k:S * (k + 1)] for k in range(4)]
        wq_c = [wq16[:, C * k:C * (k + 1)] for k in range(4)]
        wkv_c = [wkv16[:, 2 * C * k:2 * C * (k + 1)] for k in range(4)]
        wp_c = [wp16[:, C * k:C * (k + 1)] for k in range(4)]
        wcs_c = [wcs16[:, C * k:C * (k + 1)] for k in range(4)]
        wcc_c = [wcc16[:, 128 * k:128 * (k + 1)] for k in range(4)]

        kT16 = [main.tile([128, S], bf16, tag=f"kt{j}", name=f"kt{j}")
                for j in range(4)]
        qT16 = [main.tile([128, S], bf16, tag=f"qt{j}", name=f"qt{j}")
                for j in range(4)]
        # v pair tiles: [128 keys, (t2=2, h=8, e=128)] fp8e4; e=0 is the 1.0
        # ones column (Z lands on o row 0), e=64:128 the v data, 1:64 zeros.
        vp = [main.tile([128, 2 * 8 * 128], fp8, tag=f"vp{p}", name=f"vp{p}")
              for p in range(4)]
        rtb = [main.tile([128, S], bf16, tag=f"rt{j}", name=f"rt{j}")
               for j in range(4)]
        outcf_sb = main.tile([128, S], bf16, tag="ocf", name="ocf")

        # ---- DMA emission order: attention-critical first ----
        def dma_merged(dst_tile, src_ap, k, lo=0, hi=None):
            hi = k if hi is None else hi
            src3 = src_ap.rearrange("(k p) s -> p k s", k=k)
            dst3 = dst_tile[:].rearrange("p (k s) -> p k s", k=k)
            nc.sync.dma_start(dst3[:, lo:hi, :], src3[:, lo:hi, :])

        def dma_cols(dst_tile, src_ap, k, c0, c1):
            # column-group slice across all k row-chunks in one strided DMA
            src3 = src_ap.rearrange("(k p) s -> p k s", k=k)
            dst3 = dst_tile[:].rearrange("p (k s) -> p k s", k=k)
            nc.sync.dma_start(dst3[:, :, c0:c1], src3[:, :, c0:c1])

        # critical path for the first score matmul: comp + ctok + the j0
        # column groups of Wk/Wq (~2.25MB); everything else after
        dma_merged(comp16, comp16d, 4, 0, 2)
        dma_cols(wkv16, wkv16d, 4, 0, 128)        # Wk j0 cols
        dma_merged(comp16, comp16d, 4, 2, 4)
        dma_merged(ctok16, ctok16d, 4, 0, 2)
        dma_cols(wq16, wq16d, 4, 0, 128)          # Wq j0 cols
        dma_merged(ctok16, ctok16d, 4, 2, 4)
        nc.sync.dma_start(g_sb[:], gated[:])
        dma_cols(wkv16, wkv16d, 4, 512, 1024)     # V half (v jobs, early)
        dma_cols(wkv16, wkv16d, 4, 128, 512)      # Wk j1-3
        dma_cols(wq16, wq16d, 4, 128, 512)        # Wq j1-3
        dma_merged(cf16, cf16d, 4)
        dma_merged(wcc16, wcc16d, 4)
        dma_merged(wp16, wp16d, 4)
        dma_merged(wcs16, wcs16d, 4)

        # ones + zero-pad columns of the v tiles (SBUF memset = Pool)
        warm_src = main.tile([128, 128], bf16, tag="warm", name="warm")
        nc.gpsimd.memset(warm_src[:], 0.25)
        # preload the Exp activation table (1.28us) off the critical stream
        dummy_pt = main.tile([1, 8], fp8e5, tag="dpt", name="dpt")
        nc.scalar.activation(dummy_pt[0:1, :], warm_src[0:1, 0:8], EXP,
                             bias=ebias[0:1, 0:1], scale=SCALE)

        for p in range(4):
            vv = vp[p][:].rearrange("p (t h e) -> p t h e", t=2, h=8)
            nc.gpsimd.memset(vv[:, :, :, 0:1], 1.0)
            nc.gpsimd.memset(vv[:, :, :, 1:64], 0.0)

        with tc.tile_pool(name="psMain", bufs=1, space="PSUM") as ps:
            # p-state spin-up gated on the first comp chunk so the ramp is
            # still warm when the first kq matmuls run (dep-free warms would
            # finish during the DMA wait and let the clock reset)
            for _ in range(28):
                wtp = ps.tile([128, 1024], f32, tag="sc", bufs=3)
                nc.tensor.matmul(wtp[:, 0:128], warm_src[:],
                                 comp16[:, 0:128], start=True, stop=True)

            # ---- QKV projection half-jobs (bf16, ~2048 PE cycles each) ----
            kq_acc = {}

            def emit_kq_half(dst, w_c_, x_c_, j, t):
                if (id(dst), j) not in kq_acc:
                    kq_acc[(id(dst), j)] = ps.tile(
                        [128, 1024], f32, tag="sc", bufs=3, name=f"kqa{j}")
                acc = kq_acc[(id(dst), j)]
                for k in range(4):
                    nc.tensor.matmul(
                        acc[:, 512 * t:512 * (t + 1)],
                        w_c_[k][:, 128 * j:128 * (j + 1)],
                        x_c_[k][:, 512 * t:512 * (t + 1)],
                        start=(k == 0), stop=(k == 3))
                nc.vector.tensor_copy(dst[:, 512 * t:512 * (t + 1)],
                                      acc[:, 512 * t:512 * (t + 1)])
                if t == 1:
                    del kq_acc[(id(dst), j)]

            def emit_kq_q(dst, w_c_, x_c_, j, q):
                # quarter job: t = q//2, k-chunk pair = (q%2)*2
                t, k0 = q // 2, (q % 2) * 2
                if (id(dst), j) not in kq_acc:
                    kq_acc[(id(dst), j)] = ps.tile(
                        [128, 1024], f32, tag="sc", bufs=3, name=f"kqq{j}")
                acc = kq_acc[(id(dst), j)]
                for k in (k0, k0 + 1):
                    nc.tensor.matmul(
                        acc[:, 512 * t:512 * (t + 1)],
                        w_c_[k][:, 128 * j:128 * (j + 1)],
                        x_c_[k][:, 512 * t:512 * (t + 1)],
                        start=(k == 0), stop=(k == 3))
                if q % 2 == 1:
                    nc.vector.tensor_copy(dst[:, 512 * t:512 * (t + 1)],
                                          acc[:, 512 * t:512 * (t + 1)])
                if q == 3:
                    del kq_acc[(id(dst), j)]

            v_acc = {}

            def emit_v_half(p, t2):
                # token block 2p+t2; drain (strided, ->fp8e4) at t2==1
                if p not in v_acc:
                    v_acc[p] = ps.tile([128, 1024], f32, tag="sc", bufs=3,
                                       name=f"va{p}")
                acc = v_acc[p]
                tb = 2 * p + t2
                for k in range(4):
                    nc.tensor.matmul(
                        acc[:, 512 * t2:512 * (t2 + 1)],
                        comp_c[k][:, 128 * tb:128 * (tb + 1)],
                        wkv_c[k][:, C:2 * C],
                        start=(k == 0), stop=(k == 3))
                if t2 == 1:
                    del v_acc[p]
                    dst = vp[p][:].rearrange("p (t h e) -> p t h e", t=2, h=8)
                    src = acc[:].rearrange("p (t h d) -> p t h d", t=2, h=8)
                    nc.vector.tensor_copy(dst[:, :, :, 64:128],
                                          src[:, :, :, :])

            cf_acc = {}

            def emit_conv_cf_h(pc, half):
                if pc not in cf_acc:
                    cf_acc[pc] = ps.tile([128, 1024], f32, tag="sc", bufs=3,
                                         name=f"cfa{pc}")
                acc = cf_acc[pc]
                for g in (2 * half, 2 * half + 1):
                    nc.tensor.matmul(acc[:, 0:512], wcc_c[g][:, :],
                                     cf_c[g][:, 512 * pc:512 * (pc + 1)],
                                     start=(g == 0), stop=(g == 3))
                if half == 1:
                    del cf_acc[pc]
                    nc.vector.tensor_copy(
                        outcf_sb[:, 512 * pc:512 * (pc + 1)], acc[:, 0:512])
                    nc.sync.dma_start(
                        out_cf[:, 512 * pc:512 * (pc + 1)],
                        outcf_sb[:, 512 * pc:512 * (pc + 1)])

            def warm_on(src_cols, n):
                # spin the PE clock on matmuls gated by a late DMA chunk so
                # the busy-run is alive when the real matmuls become ready
                for _ in range(n):
                    wtp = ps.tile([128, 1024], f32, tag="sc", bufs=3)
                    nc.tensor.matmul(wtp[:, 0:128], warm_src[:], src_cols,
                                     start=True, stop=True)

            # pre-phase: just enough for sc(0,0): kT0 keys 0:512, qT0 full
            warm_on(comp16[:, 3 * S:3 * S + 128], 4)
            emit_kq_half(kT16[0], wkv_c, comp_c, 0, 0)
            warm_on(ctok16[:, 0:128], 4)
            emit_kq_half(qT16[0], wq_c, ctok_c, 0, 0)
            warm_on(ctok16[:, 3 * S:3 * S + 128], 4)
            emit_kq_half(qT16[0], wq_c, ctok_c, 0, 1)

            # interleaved half-jobs, keyed by the (head, kt) slot AFTER whose
            # score-matmuls they are emitted.  v pair p is needed by the o
            # matmul at slot (h,6)/(h,7)/(h+1,0)/(h+1,1); kT/qT j by (2j,0).
            ilv = {
                (0, 0): lambda: emit_kq_half(kT16[0], wkv_c, comp_c, 0, 1),
                (0, 1): lambda: emit_v_half(0, 0),
                (0, 2): lambda: emit_v_half(0, 1),
                (0, 3): lambda: emit_v_half(1, 0),
                (0, 4): lambda: emit_v_half(1, 1),
                (0, 5): lambda: emit_v_half(2, 0),
                (0, 6): lambda: emit_v_half(2, 1),
                (0, 7): lambda: emit_v_half(3, 0),
                (1, 0): lambda: emit_v_half(3, 1),
                (1, 1): lambda: emit_kq_q(kT16[1], wkv_c, comp_c, 1, 0),
                (1, 2): lambda: emit_kq_q(kT16[1], wkv_c, comp_c, 1, 1),
                (1, 3): lambda: emit_kq_q(kT16[1], wkv_c, comp_c, 1, 2),
                (1, 4): lambda: emit_kq_q(kT16[1], wkv_c, comp_c, 1, 3),
                (1, 5): lambda: emit_kq_half(qT16[1], wq_c, ctok_c, 1, 0),
                (1, 6): lambda: emit_kq_q(qT16[1], wq_c, ctok_c, 1, 2),
                (1, 7): lambda: emit_kq_q(qT16[1], wq_c, ctok_c, 1, 3),
                (2, 1): lambda: emit_kq_q(kT16[2], wkv_c, comp_c, 2, 0),
                (2, 2): lambda: emit_kq_q(kT16[2], wkv_c, comp_c, 2, 1),
                (2, 3): lambda: emit_kq_q(kT16[2], wkv_c, comp_c, 2, 2),
                (2, 4): lambda: emit_kq_q(kT16[2], wkv_c, comp_c, 2, 3),
                (2, 5): lambda: emit_kq_q(qT16[2], wq_c, ctok_c, 2, 0),
                (2, 6): lambda: emit_kq_q(qT16[2], wq_c, ctok_c, 2, 1),
                (2, 7): lambda: emit_kq_q(qT16[2], wq_c, ctok_c, 2, 2),
                (3, 1): lambda: emit_kq_q(qT16[2], wq_c, ctok_c, 2, 3),
                (3, 2): lambda: emit_kq_q(kT16[3], wkv_c, comp_c, 3, 0),
                (3, 3): lambda: emit_kq_q(kT16[3], wkv_c, comp_c, 3, 1),
                (3, 4): lambda: emit_kq_q(kT16[3], wkv_c, comp_c, 3, 2),
                (3, 5): lambda: emit_kq_q(kT16[3], wkv_c, comp_c, 3, 3),
                (3, 6): lambda: emit_kq_q(qT16[3], wq_c, ctok_c, 3, 0),
                (3, 7): lambda: emit_kq_q(qT16[3], wq_c, ctok_c, 3, 1),
                (4, 1): lambda: emit_kq_q(qT16[3], wq_c, ctok_c, 3, 2),
                (4, 2): lambda: emit_kq_q(qT16[3], wq_c, ctok_c, 3, 3),
                (5, 1): lambda: emit_conv_cf_h(0, 0),
                (5, 2): lambda: emit_conv_cf_h(0, 1),
                (5, 4): lambda: emit_conv_cf_h(1, 0),
                (5, 5): lambda: emit_conv_cf_h(1, 1),
            }

            # ---- attention ----
            pt_pool = {}      # (h, p) -> pt pair tile [128, 2048] e5m2
            o_tiles = {}      # h -> o psum tile

            def emit_sc(h, kt):
                jq, row = h // 2, 64 * (h % 2)
                sc = ps.tile([128, S], f32, tag="sc", bufs=3)
                for qc in range(2):
                    nc.tensor.matmul(
                        sc[:, 512 * qc:512 * (qc + 1)],
                        kT16[jq][row:row + 64, 128 * kt:128 * (kt + 1)],
                        qT16[jq][row:row + 64, 512 * qc:512 * (qc + 1)],
                        start=True, stop=True)
                if kt % 2 == 0:
                    pt_pool[(h, kt // 2)] = main.tile(
                        [128, 2048], fp8e5, tag="pt", bufs=6,
                        name=f"pt{h}_{kt // 2}")
                pt = pt_pool[(h, kt // 2)]
                nc.scalar.activation(pt[:, 1024 * (kt % 2):1024 * (kt % 2 + 1)],
                                     sc[:], EXP, bias=ebias[:, 0:1],
                                     scale=SCALE)

            def emit_o(h, p):
                # o[0,:] = Z, o[64:128,:] = P@v_h  (DoubleRow over kt pair)
                if h not in o_tiles:
                    o_tiles[h] = ps.tile([128, S], f32, tag="o", bufs=1,
                                         name=f"o{h}")
                o_ps = o_tiles[h]
                vv = vp[p][:].rearrange("p (t h e) -> p t h e", t=2, h=8)
                ptv = pt_pool.pop((h, p))[:].rearrange("p (t q) -> p t q", t=2)
                for qc in range(2):
                    nc.tensor.matmul(
                        o_ps[:, 512 * qc:512 * (qc + 1)],
                        vv[:, :, h:h + 1, :],
                        ptv[:, :, 512 * qc:512 * (qc + 1)],
                        start=(p == 0), stop=(p == 3), perf_mode=DR)

            def emit_norm(h, split=False):
                jq, row = h // 2, 64 * (h % 2)
                o_ps = o_tiles.pop(h)
                if split:
                    # qc-halved chain: shorter serial latency on the tail,
                    # and the pair-stt halves unblock the g0/g1 fins early
                    for qc in range(2):
                        zi = main.tile([1, 512], f32, tag="zis", bufs=2,
                                       name=f"zis{h}{qc}")
                        zbc = main.tile([64, 512], f32, tag="zbs", bufs=2,
                                        name=f"zbs{h}{qc}")
                        nc.vector.reciprocal_approx_fast(
                            zi[0:1, :], o_ps[0:1, 512 * qc:512 * (qc + 1)])
                        nc.gpsimd.partition_broadcast(zbc[0:64, :],
                                                      zi[0:1, :])
                        nc.vector.scalar_tensor_tensor(
                            rtb[jq][row:row + 64, 512 * qc:512 * (qc + 1)],
                            o_ps[64:128, 512 * qc:512 * (qc + 1)], 1.0,
                            zbc[0:64, :], BYPASS, MULT)
                        if h % 2 == 1:
                            nc.vector.scalar_tensor_tensor(
                                rtb[jq][:, 512 * qc:512 * (qc + 1)],
                                ctok_c[jq][:, 512 * qc:512 * (qc + 1)],
                                g_sb[:, 0:1],
                                rtb[jq][:, 512 * qc:512 * (qc + 1)],
                                MULT, ADD)
                    return
                else:
                    zi = main.tile([1, S], f32, tag="zi", bufs=2,
                                   name=f"zi{h}")
                    zbc = main.tile([64, S], f32, tag="zb", bufs=2,
                                    name=f"zb{h}")
                    nc.vector.reciprocal_approx_fast(zi[0:1, :], o_ps[0:1, :])
                    nc.gpsimd.partition_broadcast(zbc[0:64, :], zi[0:1, :])
                    nc.vector.scalar_tensor_tensor(
                        rtb[jq][row:row + 64, :], o_ps[64:128, :], 1.0,
                        zbc[0:64, :], BYPASS, MULT)
                if h % 2 == 1:  # pair complete: s_in^T = rtb + gate*ctokT
                    nc.vector.scalar_tensor_tensor(
                        rtb[jq][:], ctok_c[jq][:], g_sb[:, 0:1], rtb[jq][:],
                        MULT, ADD)

            # flat emission; o(h) trails into head h+1 per the o-bank cycle
            for h in range(NH):
                for kt in range(8):
                    emit_sc(h, kt)
                    if h > 0:
                        if kt == 0:
                            emit_o(h - 1, 2)
                        elif kt == 1:
                            emit_o(h - 1, 3)
                        elif kt == 2:
                            emit_norm(h - 1)
                    if (h, kt) in ilv:
                        ilv[(h, kt)]()
                    if kt == 6:
                        emit_o(h, 0)
                    elif kt == 7:
                        emit_o(h, 1)
            # ---- tail (same pool: proj/conv accumulators ride the
            # sc/o tags as [128,512] halves; proj j0-2 partials fill the
            # end of the exp stream before the last o matmuls) ----
            rtb_v = [rtb[j][:].rearrange("p (g i two) -> p g two i",
                                         g=4, two=2) for j in range(4)]
            s2d_sb = [[main.tile([128, 512], bf16, tag=f"s2d{par}{g}",
                                 name=f"s2d{par}{g}") for g in range(4)]
                      for par in range(2)]
            ostp = [main.tile([128, 4 * 512], bf16, tag=f"ostp{par}",
                              name=f"ostp{par}") for par in range(2)]
            pj = {}

            def emit_pre2(pairs):
                tl_ = ps.tile([128, 1024], f32, tag="sc", bufs=3,
                              name=f"pj{pairs[0][0]}{pairs[0][1]}")
                for i, (par, g) in enumerate(pairs):
                    acc = tl_[:, 512 * i:512 * (i + 1)]
                    pj[(par, g)] = acc
                    for j in range(3):
                        nc.tensor.matmul(acc, rtb_v[j][:, g, par, :],
                                         wp_c[j][:, :], start=(j == 0),
                                         stop=False)

            def emit_fin(par, g, eng):
                acc = pj.pop((par, g), None)
                if acc is None:
                    # not pre-started: full 4-chain into the o-tag tile half
                    if "otl" not in pj:
                        pj["otl"] = ps.tile([128, 1024], f32, tag="o",
                                            bufs=1, name="pjo")
                    acc = pj["otl"][:, 512 * (g % 2):512 * (g % 2 + 1)]
                    for j in range(4):
                        nc.tensor.matmul(acc, rtb_v[j][:, g, par, :],
                                         wp_c[j][:, :], start=(j == 0),
                                         stop=(j == 3))
                else:
                    nc.tensor.matmul(acc, rtb_v[3][:, g, par, :],
                                     wp_c[3][:, :], start=False, stop=True)
                eng(s2d_sb[par][g][:], acc)

            def emit_conv_s(par):
                for ocp in range(2):
                    tl_ = ps.tile([128, 1024], f32, tag="sc", bufs=3,
                                  name=f"cv{par}{ocp}")
                    for i in range(2):
                        oc = 2 * ocp + i
                        acc = tl_[:, 512 * i:512 * (i + 1)]
                        for g in range(4):
                            nc.tensor.matmul(
                                acc, wcs_c[g][:, 128 * oc:128 * (oc + 1)],
                                s2d_sb[par][g][:],
                                start=(g == 0), stop=(g == 3))
                        eng = (nc.vector.tensor_copy if oc % 2 == 0
                               else nc.scalar.copy)
                        eng(ostp[par][:, 512 * oc:512 * (oc + 1)], acc)
                    out3 = out_p.rearrange("(oc p) s -> p oc s", oc=4)
                    src3 = ostp[par][:].rearrange("p (oc s) -> p oc s", oc=4)
                    nc.sync.dma_start(
                        out3[:, 2 * ocp:2 * ocp + 2,
                             512 * par:512 * (par + 1)],
                        src3[:, 2 * ocp:2 * ocp + 2, :])

            emit_pre2([(0, 0), (0, 1)])
            emit_pre2([(0, 2), (0, 3)])
            emit_o(NH - 1, 2)
            emit_pre2([(1, 0), (1, 1)])
            emit_o(NH - 1, 3)
            emit_norm(NH - 1, split=True)
            # fins for s2d column half 0 (g0,g1) only need the qc0 half of
            # the last chain; g2,g3 the qc1 half
            for par, g in [(0, 0), (0, 1), (1, 0), (1, 1),
                           (0, 2), (0, 3), (1, 2), (1, 3)]:
                emit_fin(par, g, nc.vector.tensor_copy if g % 2 == 0
                         else nc.scalar.copy)
            emit_conv_s(0)
            emit_conv_s(1)

    nc.compile()
    _CACHE["nc"] = nc
    return nc


def _shard_inputs(content_feat, components, pos_emb, Wq, Wkv, Wproj, bproj,
                  Wconv, bconv):
    import ml_dtypes

    bf = ml_dtypes.bfloat16
    f = np.float32
    pos2 = np.asarray(pos_emb, dtype=f).reshape(S, C)
    wq16 = np.asarray(Wq, dtype=f).astype(bf)
    wkv16 = np.asarray(Wkv, dtype=f).astype(bf)
    wp16 = np.asarray(Wproj, dtype=f).astype(bf)
    wcT = np.ascontiguousarray(np.asarray(Wconv, dtype=f).T)
    wcs16 = np.ascontiguousarray(wcT[:C]).astype(bf)
    in_maps = []
    for core in range(N_CORES):
        b, n = core // 4, core % 4
        ctokT = np.ascontiguousarray(
            (np.asarray(content_feat[b], dtype=f).reshape(S, C) + pos2).T)
        compT = np.ascontiguousarray(
            (np.asarray(components[n, b], dtype=f).reshape(S, C) + pos2).T)
        in_maps.append({
            "comp16": compT.astype(bf),
            "ctok16": ctokT.astype(bf),
            "cf16": np.ascontiguousarray(
                np.asarray(content_feat[b], dtype=f).reshape(C, S)).astype(bf),
            "wq16": wq16,
            "wkv16": wkv16,
            "wp16": wp16,
            "wcs16": wcs16,
            "wcc16": np.ascontiguousarray(
                wcT[C:, 128 * n:128 * (n + 1)]).astype(bf),
            "gate": np.full((128, 1), 1.0 if n == 0 else 0.0, dtype=f),
        })
    return in_maps


def _run(trace=False, **inputs):
    from concourse.bass_utils import run_bass_kernel_spmd

    nc = _build()
    in_maps = _shard_inputs(**inputs)
    res = run_bass_kernel_spmd(nc, in_maps, list(range(N_CORES)), trace=trace)
    outs = [np.asarray(res.results[i]["out_p"], dtype=np.float64)
            for i in range(N_CORES)]
    out = np.stack([outs[0] + outs[1] + outs[2] + outs[3],
                    outs[4] + outs[5] + outs[6] + outs[7]], axis=0)
    for core in range(N_CORES):
        b, n = core // 4, core % 4
        out[b, 128 * n:128 * (n + 1), :] += np.asarray(
            res.results[core]["out_cf"], dtype=np.float64)
    # host-side affine constants: out += bconv[o] + ws[o]*bproj[i%512]
    # with ws[o] = sum_c Wconv[o, c<C]  (bproj enters via the conv s-half)
    Wconv = np.asarray(inputs["Wconv"], dtype=np.float64)
    bproj = np.asarray(inputs["bproj"], dtype=np.float64)
    bconv = np.asarray(inputs["bconv"], dtype=np.float64)
    ws = Wconv[:, :C].sum(axis=1)
    bias_img = bconv[:, None] + np.outer(ws, np.concatenate([bproj, bproj]))
    out += bias_img[None, :, :]
    return out.reshape(B, C, H, W).astype(np.float32), res


def kernel(**inputs):
    out, _ = _run(trace=False, **inputs)
    return out


# revision 20
# speedup vs baseline: 1.2152x; 1.0042x over previous
"""Trainium2 Bass kernel for nn_Attention_54391465836966.

Math (per batch b, component n; one (b,n) pair per core, host sums over n):
  ctok = content_feat[b].raw_reshape(S,C) + pos          # [1024, 512]
  comp_tok = components[n,b].raw_reshape(S,C) + pos
  q = ctok @ Wq ; k,v = comp_tok @ Wkv (split)
  per head h: P = exp(scale q_h k_h^T); o_h = (P @ v_h) / rowsum(P)
  s_in = concat_h(o_h) + gate*ctok                        # gate = [n==0]
  s2d  = raw_reshape(s_in @ Wproj)                        # bias via host
  out  = Wconv^T[:C] @ s2d + Wconv^T[C:] @ cf             # + bias via host

Design notes (cost-model-driven):
- The P@V (o) matmuls run fp8 DoubleRow (2 k-tiles per instruction at 0.5
  cycles/row = 4x the bf16 rate): pt = exp output in e5m2 (its 22-efold
  dynamic range covers the unnormalized exp spread -- score sigma ~2.0 ->
  +-11 efolds; e4m3's 11.7 efolds would overflow), v drained to e4m3.
  Attention-output noise from fp8 is diluted by the ctok residual and
  P-quantization partially cancels against Z (measured 1.4e-2 total).
- The q/k path stays bf16 end-to-end: fp8 anywhere before the exp adds
  ~0.17 absolute score noise -> 17% P-reweighting with only ~19 effective
  keys after softmax (score sigma 2.0) -> 3-4e-2 output error (measured).
- exp writes pt e5m2 directly with a constant -4 bias (normalization
  cancels it exactly; caps pt at exp(score-4) with overflow only past
  7.5 sigma scores, flush below -7.1 negligible).
- Z rides row 0 of the o PSUM tile (ones column first in v): the custom-DVE
  reciprocal reads partition 0 of PSUM directly (hw quirk: it ignores the
  AP's partition base), skipping the per-head z-copy entirely.  Drain +
  normalize fuse into one scalar_tensor_tensor from PSUM (v data sits at
  e=64:128 so the o data rows land at the 32-aligned partition base 64).
- The projection uses stride-2 column slices of s_in^T as the stationary
  operand, which makes its PSUM output land directly in the s->s2d
  raw-reshape layout: no PE transposes anywhere in the kernel.
- bproj/bconv are affine constants independent of the data path; their
  contribution is bconv[o] + (sum_c Wconv[o,c<C]) * bproj[i%512], a rank-1
  image the host adds during unshard.
- PSUM: score tiles tag "sc" [128,1024] bufs=3 (6 banks) shared by the
  QKV-projection accumulations (pre-phase + interleaved jobs borrow
  rotation slots); o tag [128,1024] bufs=1 (2 banks).  GPSIMD cannot touch
  PSUM, so all PSUM drains go to DVE (ACT helps only post-exp in the tail).
- ACT runs exps only (64 x [128,1024] ~= 1.04us each); QKV/conv_cf jobs are
  split into ~2048-cycle half-jobs and interleaved between score matmuls so
  the exp stream never starves.  o matmuls for head h trail into head h+1's
  first slots; the recip->broadcast->normalize chain then frees the single
  o buffer before head h+1 needs it.
"""
import sys

sys.path.insert(0, "/opt/trn_rl_repo")

import numpy as np

N_CORES = 8
B, C, H, W = 2, 512, 32, 32
S = H * W  # 1024
NH, HD = 8, 64
SCALE = HD ** -0.5
EXP_BIAS = -4.0

_CACHE = {}


def _build():
    if "nc" in _CACHE:
        return _CACHE["nc"]
    from contextlib import ExitStack

    import concourse.bacc as bacc
    import concourse.mybir as mybir
    import concourse.tile as tile

    f32 = mybir.dt.float32
    bf16 = mybir.dt.bfloat16
    fp8 = mybir.dt.float8e4
    fp8e5 = mybir.dt.float8e5
    EXP = mybir.ActivationFunctionType.Exp
    MULT = mybir.AluOpType.mult
    ADD = mybir.AluOpType.add
    BYPASS = mybir.AluOpType.bypass
    DR = mybir.MatmulPerfMode.DoubleRow

    nc = bacc.Bacc("TRN2", target_bir_lowering=False, debug=False,
                   num_devices=N_CORES)

    din = lambda n, s, dt: nc.dram_tensor(n, s, dt, kind="ExternalInput").ap()
    comp16d = din("comp16", [C, S], bf16)   # (comp_tok + pos).T
    ctok16d = din("ctok16", [C, S], bf16)   # (content_tok + pos).T
    cf16d = din("cf16", [C, S], bf16)       # content_feat[b] raw [C,S]
    wq16d = din("wq16", [C, C], bf16)       # Wq
    wkv16d = din("wkv16", [C, 2 * C], bf16)  # Wkv (cols 0:C K, C:2C V)
    wp16d = din("wp16", [C, C], bf16)       # Wproj
    wcs16d = din("wcs16", [C, C], bf16)     # Wconv.T rows :C (s half)
    wcc16d = din("wcc16", [C, 128], bf16)   # Wconv.T[C:, 128n:128(n+1)]
    gated = din("gate", [128, 1], f32)      # 1.0 on n==0 cores else 0.0
    out_p = nc.dram_tensor("out_p", [C, S], bf16, kind="ExternalOutput").ap()
    out_cf = nc.dram_tensor("out_cf", [128, S], bf16,
                            kind="ExternalOutput").ap()

    with tile.TileContext(nc) as tc, ExitStack() as ctx:
        main = ctx.enter_context(tc.tile_pool(name="main", bufs=1))

        g_sb = main.tile([128, 1], f32, tag="g")
        ebias = main.tile([128, 1], f32, tag="eb")
        nc.gpsimd.memset(ebias[:], EXP_BIAS)

        # ---- persistent SBUF tiles (one merged DMA per DRAM tensor) ----
        comp16 = main.tile([128, 4 * S], bf16, tag="cm16", name="cm16")
        ctok16 = main.tile([128, 4 * S], bf16, tag="ct16", name="ct16")
        cf16 = main.tile([128, 4 * S], bf16, tag="cf16", name="cf16")
        wq16 = main.tile([128, 4 * C], bf16, tag="wq16", name="wq16")
        wkv16 = main.tile([128, 4 * 2 * C], bf16, tag="wkv16", name="wkv16")
        wp16 = main.tile([128, 4 * C], bf16, tag="wp16", name="wp16")
        wcs16 = main.tile([128, 4 * C], bf16, tag="wcs16", name="wcs16")
        wcc16 = main.tile([128, 4 * 128], bf16, tag="wcc16", name="wcc16")

        comp_c = [comp16[:, S * k:S * (k + 1)] for k in range(4)]
        ctok_c = [ctok16[:, S * k:S * (k + 1)] for k in range(4)]
        cf_c = [cf16[:, S * k:S * (k + 1)] for k in range(4)]
        wq_c = [wq16[:, C * k:C * (k + 1)] for k in range(4)]
        wkv_c = [wkv16[:, 2 * C * k:2 * C * (k + 1)] for k in range(4)]
        wp_c = [wp16[:, C * k:C * (k + 1)] for k in range(4)]
        wcs_c = [wcs16[:, C * k:C * (k + 1)] for k in range(4)]
        wcc_c = [wcc16[:, 128 * k:128 * (k + 1)] for k in range(4)]

        kT16 = [main.tile([128, S], bf16, tag=f"kt{j}", name=f"kt{j}")
                for j in range(4)]
        qT16 = [main.tile([128, S], bf16, tag=f"qt{j}", name=f"qt{j}")
                for j in range(4)]
        # v pair tiles: [128 keys, (t2=2, h=8, e=128)] fp8e4; e=0 is the 1.0
        # ones column (Z lands on o row 0), e=64:128 the v data, 1:64 zeros.
        vp = [main.tile([128, 2 * 8 * 128], fp8, tag=f"vp{p}", name=f"vp{p}")
              for p in range(4)]
        rtb = [main.tile([128, S], bf16, tag=f"rt{j}", name=f"rt{j}")
               for j in range(4)]
        outcf_sb = main.tile([128, S], bf16, tag="ocf", name="ocf")

        # ---- DMA emission order: attention-critical first ----
        def dma_merged(dst_tile, src_ap, k, lo=0, hi=None):
            hi = k if hi is None else hi
            src3 = src_ap.rearrange("(k p) s -> p k s", k=k)
            dst3 = dst_tile[:].rearrange("p (k s) -> p k s", k=k)
            nc.sync.dma_start(dst3[:, lo:hi, :], src3[:, lo:hi, :])

        def dma_cols(dst_tile, src_ap, k, c0, c1):
            # column-group slice across all k row-chunks in one strided DMA
            src3 = src_ap.rearrange("(k p) s -> p k s", k=k)
            dst3 = dst_tile[:].rearrange("p (k s) -> p k s", k=k)
            nc.sync.dma_start(dst3[:, :, c0:c1], src3[:, :, c0:c1])

        # critical path for the first score matmul: comp + ctok + the j0
        # column groups of Wk/Wq (~2.25MB); everything else after
        dma_merged(comp16, comp16d, 4, 0, 2)
        dma_cols(wkv16, wkv16d, 4, 0, 128)        # Wk j0 cols
        dma_merged(comp16, comp16d, 4, 2, 4)
        dma_merged(ctok16, ctok16d, 4, 0, 2)
        dma_cols(wq16, wq16d, 4, 0, 128)          # Wq j0 cols
        dma_merged(ctok16, ctok16d, 4, 2, 4)
        nc.sync.dma_start(g_sb[:], gated[:])
        dma_cols(wkv16, wkv16d, 4, 512, 1024)     # V half (v jobs, early)
        dma_cols(wkv16, wkv16d, 4, 128, 512)      # Wk j1-3
        dma_cols(wq16, wq16d, 4, 128, 512)        # Wq j1-3
        dma_merged(cf16, cf16d, 4)
        dma_merged(wcc16, wcc16d, 4)
        dma_merged(wp16, wp16d, 4)
        dma_merged(wcs16, wcs16d, 4)

        # ones + zero-pad columns of the v tiles (SBUF memset = Pool)
        warm_src = main.tile([128, 128], bf16, tag="warm", name="warm")
        nc.gpsimd.memset(warm_src[:], 0.25)
        # preload the Exp activation table (1.28us) off the critical stream
        dummy_pt = main.tile([1, 8], fp8e5, tag="dpt", name="dpt")
        nc.scalar.activation(dummy_pt[0:1, :], warm_src[0:1, 0:8], EXP,
                             bias=ebias[0:1, 0:1], scale=SCALE)

        for p in range(4):
            vv = vp[p][:].rearrange("p (t h e) -> p t h e", t=2, h=8)
            nc.gpsimd.memset(vv[:, :, :, 0:1], 1.0)
            nc.gpsimd.memset(vv[:, :, :, 1:64], 0.0)

        with tc.tile_pool(name="psMain", bufs=1, space="PSUM") as ps:
            # p-state spin-up gated on the first comp chunk so the ramp is
            # still warm when the first kq matmuls run (dep-free warms would
            # finish during the DMA wait and let the clock reset)
            for _ in range(28):
                wtp = ps.tile([128, 1024], f32, tag="sc", bufs=3)
                nc.tensor.matmul(wtp[:, 0:128], warm_src[:],
                                 comp16[:, 0:128], start=True, stop=True)

            # ---- QKV projection half-jobs (bf16, ~2048 PE cycles each) ----
            kq_acc = {}

            def emit_kq_half(dst, w_c_, x_c_, j, t):
                if (id(dst), j) not in kq_acc:
                    kq_acc[(id(dst), j)] = ps.tile(
                        [128, 1024], f32, tag="sc", bufs=3, name=f"kqa{j}")
                acc = kq_acc[(id(dst), j)]
                for k in range(4):
                    nc.tensor.matmul(
                        acc[:, 512 * t:512 * (t + 1)],
                        w_c_[k][:, 128 * j:128 * (j + 1)],
                        x_c_[k][:, 512 * t:512 * (t + 1)],
                        start=(k == 0), stop=(k == 3))
                nc.vector.tensor_copy(dst[:, 512 * t:512 * (t + 1)],
                                      acc[:, 512 * t:512 * (t + 1)])
                if t == 1:
                    del kq_acc[(id(dst), j)]

            def emit_kq_q(dst, w_c_, x_c_, j, q):
                # quarter job: t = q//2, k-chunk pair = (q%2)*2
                t, k0 = q // 2, (q % 2) * 2
                if (id(dst), j) not in kq_acc:
                    kq_acc[(id(dst), j)] = ps.tile(
                        [128, 1024], f32, tag="sc", bufs=3, name=f"kqq{j}")
                acc = kq_acc[(id(dst), j)]
                for k in (k0, k0 + 1):
                    nc.tensor.matmul(
                        acc[:, 512 * t:512 * (t + 1)],
                        w_c_[k][:, 128 * j:128 * (j + 1)],
                        x_c_[k][:, 512 * t:512 * (t + 1)],
                        start=(k == 0), stop=(k == 3))
                if q % 2 == 1:
                    nc.vector.tensor_copy(dst[:, 512 * t:512 * (t + 1)],
                                          acc[:, 512 * t:512 * (t + 1)])
                if q == 3:
                    del kq_acc[(id(dst), j)]

            v_acc = {}

            def emit_v_half(p, t2):
                # token block 2p+t2; drain (strided, ->fp8e4) at t2==1.
                # pairs 0-2 accumulate in the o-tag tile (idle until the
                # first o matmul) to keep the sc rotation free for scores.
                if p not in v_acc:
                    v_acc[p] = ps.tile([128, 1024], f32,
                                       tag="o" if p < 3 else "sc",
                                       bufs=1 if p < 3 else 3,
                                       name=f"va{p}")
                acc = v_acc[p]
                tb = 2 * p + t2
                for k in range(4):
                    nc.tensor.matmul(
                        acc[:, 512 * t2:512 * (t2 + 1)],
                        comp_c[k][:, 128 * tb:128 * (tb + 1)],
                        wkv_c[k][:, C:2 * C],
                        start=(k == 0), stop=(k == 3))
                if t2 == 1:
                    del v_acc[p]
                    dst = vp[p][:].rearrange("p (t h e) -> p t h e", t=2, h=8)
                    src = acc[:].rearrange("p (t h d) -> p t h d", t=2, h=8)
                    nc.vector.tensor_copy(dst[:, :, :, 64:128],
                                          src[:, :, :, :])

            cf_acc = {}

            def emit_conv_cf_h(pc, half):
                if pc not in cf_acc:
                    cf_acc[pc] = ps.tile([128, 1024], f32, tag="sc", bufs=3,
                                         name=f"cfa{pc}")
                acc = cf_acc[pc]
                for g in (2 * half, 2 * half + 1):
                    nc.tensor.matmul(acc[:, 0:512], wcc_c[g][:, :],
                                     cf_c[g][:, 512 * pc:512 * (pc + 1)],
                                     start=(g == 0), stop=(g == 3))
                if half == 1:
                    del cf_acc[pc]
                    nc.vector.tensor_copy(
                        outcf_sb[:, 512 * pc:512 * (pc + 1)], acc[:, 0:512])
                    nc.sync.dma_start(
                        out_cf[:, 512 * pc:512 * (pc + 1)],
                        outcf_sb[:, 512 * pc:512 * (pc + 1)])

            def warm_on(src_cols, n):
                # spin the PE clock on matmuls gated by a late DMA chunk so
                # the busy-run is alive when the real matmuls become ready
                for _ in range(n):
                    wtp = ps.tile([128, 1024], f32, tag="sc", bufs=3)
                    nc.tensor.matmul(wtp[:, 0:128], warm_src[:], src_cols,
                                     start=True, stop=True)

            # pre-phase: just enough for sc(0,0): kT0 keys 0:512, qT0 full
            warm_on(comp16[:, 3 * S:3 * S + 128], 4)
            emit_kq_half(kT16[0], wkv_c, comp_c, 0, 0)
            warm_on(ctok16[:, 0:128], 4)
            emit_kq_half(qT16[0], wq_c, ctok_c, 0, 0)
            warm_on(ctok16[:, 3 * S:3 * S + 128], 4)
            emit_kq_half(qT16[0], wq_c, ctok_c, 0, 1)

            # interleaved half-jobs, keyed by the (head, kt) slot AFTER whose
            # score-matmuls they are emitted.  v pair p is needed by the o
            # matmul at slot (h,6)/(h,7)/(h+1,0)/(h+1,1); kT/qT j by (2j,0).
            ilv = {
                (0, 0): lambda: emit_kq_half(kT16[0], wkv_c, comp_c, 0, 1),
                (0, 1): lambda: emit_v_half(0, 0),
                (0, 2): lambda: emit_v_half(0, 1),
                (0, 3): lambda: emit_v_half(1, 0),
                (0, 4): lambda: emit_v_half(1, 1),
                (0, 5): lambda: emit_v_half(2, 0),
                (0, 6): lambda: emit_v_half(2, 1),
                (0, 7): lambda: emit_v_half(3, 0),
                (1, 0): lambda: emit_v_half(3, 1),
                (1, 1): lambda: emit_kq_q(kT16[1], wkv_c, comp_c, 1, 0),
                (1, 2): lambda: emit_kq_q(kT16[1], wkv_c, comp_c, 1, 1),
                (1, 3): lambda: emit_kq_q(kT16[1], wkv_c, comp_c, 1, 2),
                (1, 4): lambda: emit_kq_q(kT16[1], wkv_c, comp_c, 1, 3),
                (1, 5): lambda: emit_kq_half(qT16[1], wq_c, ctok_c, 1, 0),
                (1, 6): lambda: emit_kq_q(qT16[1], wq_c, ctok_c, 1, 2),
                (1, 7): lambda: emit_kq_q(qT16[1], wq_c, ctok_c, 1, 3),
                (2, 1): lambda: emit_kq_q(kT16[2], wkv_c, comp_c, 2, 0),
                (2, 2): lambda: emit_kq_q(kT16[2], wkv_c, comp_c, 2, 1),
                (2, 3): lambda: emit_kq_q(kT16[2], wkv_c, comp_c, 2, 2),
                (2, 4): lambda: emit_kq_q(kT16[2], wkv_c, comp_c, 2, 3),
                (2, 5): lambda: emit_kq_q(qT16[2], wq_c, ctok_c, 2, 0),
                (2, 6): lambda: emit_kq_q(qT16[2], wq_c, ctok_c, 2, 1),
                (2, 7): lambda: emit_kq_q(qT16[2], wq_c, ctok_c, 2, 2),
                (3, 1): lambda: emit_kq_q(qT16[2], wq_c, ctok_c, 2, 3),
                (3, 2): lambda: emit_kq_q(kT16[3], wkv_c, comp_c, 3, 0),
                (3, 3): lambda: emit_kq_q(kT16[3], wkv_c, comp_c, 3, 1),
                (3, 4): lambda: emit_kq_q(kT16[3], wkv_c, comp_c, 3, 2),
                (3, 5): lambda: emit_kq_q(kT16[3], wkv_c, comp_c, 3, 3),
                (3, 6): lambda: emit_kq_q(qT16[3], wq_c, ctok_c, 3, 0),
                (3, 7): lambda: emit_kq_q(qT16[3], wq_c, ctok_c, 3, 1),
                (4, 1): lambda: emit_kq_q(qT16[3], wq_c, ctok_c, 3, 2),
                (4, 2): lambda: emit_kq_q(qT16[3], wq_c, ctok_c, 3, 3),
                (5, 1): lambda: emit_conv_cf_h(0, 0),
                (5, 2): lambda: emit_conv_cf_h(0, 1),
                (5, 4): lambda: emit_conv_cf_h(1, 0),
                (5, 5): lambda: emit_conv_cf_h(1, 1),
            }

            # ---- attention ----
            pt_pool = {}      # (h, p) -> pt pair tile [128, 2048] e5m2
            o_tiles = {}      # h -> o psum tile

            def emit_sc(h, kt):
                jq, row = h // 2, 64 * (h % 2)
                sc = ps.tile([128, S], f32, tag="sc", bufs=3)
                for qc in range(2):
                    nc.tensor.matmul(
                        sc[:, 512 * qc:512 * (qc + 1)],
                        kT16[jq][row:row + 64, 128 * kt:128 * (kt + 1)],
                        qT16[jq][row:row + 64, 512 * qc:512 * (qc + 1)],
                        start=True, stop=True)
                if kt % 2 == 0:
                    pt_pool[(h, kt // 2)] = main.tile(
                        [128, 2048], fp8e5, tag="pt", bufs=6,
                        name=f"pt{h}_{kt // 2}")
                pt = pt_pool[(h, kt // 2)]
                nc.scalar.activation(pt[:, 1024 * (kt % 2):1024 * (kt % 2 + 1)],
                                     sc[:], EXP, bias=ebias[:, 0:1],
                                     scale=SCALE)

            def emit_o(h, p):
                # o[0,:] = Z, o[64:128,:] = P@v_h  (DoubleRow over kt pair)
                if h not in o_tiles:
                    o_tiles[h] = ps.tile([128, S], f32, tag="o", bufs=1,
                                         name=f"o{h}")
                o_ps = o_tiles[h]
                vv = vp[p][:].rearrange("p (t h e) -> p t h e", t=2, h=8)
                ptv = pt_pool.pop((h, p))[:].rearrange("p (t q) -> p t q", t=2)
                for qc in range(2):
                    nc.tensor.matmul(
                        o_ps[:, 512 * qc:512 * (qc + 1)],
                        vv[:, :, h:h + 1, :],
                        ptv[:, :, 512 * qc:512 * (qc + 1)],
                        start=(p == 0), stop=(p == 3), perf_mode=DR)

            def emit_norm(h, split=False):
                jq, row = h // 2, 64 * (h % 2)
                o_ps = o_tiles.pop(h)
                if split:
                    # qc-halved chain: shorter serial latency on the tail,
                    # and the pair-stt halves unblock the g0/g1 fins early
                    for qc in range(2):
                        zi = main.tile([1, 512], f32, tag="zis", bufs=2,
                                       name=f"zis{h}{qc}")
                        zbc = main.tile([64, 512], f32, tag="zbs", bufs=2,
                                        name=f"zbs{h}{qc}")
                        nc.vector.reciprocal_approx_fast(
                            zi[0:1, :], o_ps[0:1, 512 * qc:512 * (qc + 1)])
                        nc.gpsimd.partition_broadcast(zbc[0:64, :],
                                                      zi[0:1, :])
                        nc.vector.scalar_tensor_tensor(
                            rtb[jq][row:row + 64, 512 * qc:512 * (qc + 1)],
                            o_ps[64:128, 512 * qc:512 * (qc + 1)], 1.0,
                            zbc[0:64, :], BYPASS, MULT)
                        if h % 2 == 1:
                            nc.vector.scalar_tensor_tensor(
                                rtb[jq][:, 512 * qc:512 * (qc + 1)],
                                ctok_c[jq][:, 512 * qc:512 * (qc + 1)],
                                g_sb[:, 0:1],
                                rtb[jq][:, 512 * qc:512 * (qc + 1)],
                                MULT, ADD)
                    return
                else:
                    zi = main.tile([1, S], f32, tag="zi", bufs=2,
                                   name=f"zi{h}")
                    zbc = main.tile([64, S], f32, tag="zb", bufs=2,
                                    name=f"zb{h}")
                    nc.vector.reciprocal_approx_fast(zi[0:1, :], o_ps[0:1, :])
                    nc.gpsimd.partition_broadcast(zbc[0:64, :], zi[0:1, :])
                    nc.vector.scalar_tensor_tensor(
                        rtb[jq][row:row + 64, :], o_ps[64:128, :], 1.0,
                        zbc[0:64, :], BYPASS, MULT)
                if h % 2 == 1:  # pair complete: s_in^T = rtb + gate*ctokT
                    nc.vector.scalar_tensor_tensor(
                        rtb[jq][:], ctok_c[jq][:], g_sb[:, 0:1], rtb[jq][:],
                        MULT, ADD)

            # flat emission; o(h) trails into head h+1 per the o-bank cycle
            for h in range(NH):
                for kt in range(8):
                    emit_sc(h, kt)
                    if h > 0:
                        if kt == 0:
                            emit_o(h - 1, 2)
                        elif kt == 1:
                            emit_o(h - 1, 3)
                        elif kt == 2:
                            emit_norm(h - 1)
                    if (h, kt) in ilv:
                        ilv[(h, kt)]()
                    if kt == 6:
                        emit_o(h, 0)
                    elif kt == 7:
                        emit_o(h, 1)
            # ---- tail (same pool: proj/conv accumulators ride the
            # sc/o tags as [128,512] halves; proj j0-2 partials fill the
            # end of the exp stream before the last o matmuls) ----
            rtb_v = [rtb[j][:].rearrange("p (g i two) -> p g two i",
                                         g=4, two=2) for j in range(4)]
            s2d_sb = [[main.tile([128, 512], bf16, tag=f"s2d{par}{g}",
                                 name=f"s2d{par}{g}") for g in range(4)]
                      for par in range(2)]
            ostp = [main.tile([128, 4 * 512], bf16, tag=f"ostp{par}",
                              name=f"ostp{par}") for par in range(2)]
            pj = {}

            def emit_pre2(pairs):
                tl_ = ps.tile([128, 1024], f32, tag="sc", bufs=3,
                              name=f"pj{pairs[0][0]}{pairs[0][1]}")
                for i, (par, g) in enumerate(pairs):
                    acc = tl_[:, 512 * i:512 * (i + 1)]
                    pj[(par, g)] = acc
                    for j in range(3):
                        nc.tensor.matmul(acc, rtb_v[j][:, g, par, :],
                                         wp_c[j][:, :], start=(j == 0),
                                         stop=False)

            def emit_fin(par, g, eng):
                acc = pj.pop((par, g), None)
                if acc is None:
                    # not pre-started: full 4-chain into the o-tag tile half
                    if "otl" not in pj:
                        pj["otl"] = ps.tile([128, 1024], f32, tag="o",
                                            bufs=1, name="pjo")
                    acc = pj["otl"][:, 512 * (g % 2):512 * (g % 2 + 1)]
                    for j in range(4):
                        nc.tensor.matmul(acc, rtb_v[j][:, g, par, :],
                                         wp_c[j][:, :], start=(j == 0),
                                         stop=(j == 3))
                else:
                    nc.tensor.matmul(acc, rtb_v[3][:, g, par, :],
                                     wp_c[3][:, :], start=False, stop=True)
                eng(s2d_sb[par][g][:], acc)

            def emit_conv_s(par):
                for ocp in range(2):
                    tl_ = ps.tile([128, 1024], f32, tag="sc", bufs=3,
                                  name=f"cv{par}{ocp}")
                    for i in range(2):
                        oc = 2 * ocp + i
                        acc = tl_[:, 512 * i:512 * (i + 1)]
                        for g in range(4):
                            nc.tensor.matmul(
                                acc, wcs_c[g][:, 128 * oc:128 * (oc + 1)],
                                s2d_sb[par][g][:],
                                start=(g == 0), stop=(g == 3))
                        eng = (nc.vector.tensor_copy if oc % 2 == 0
                               else nc.scalar.copy)
                        eng(ostp[par][:, 512 * oc:512 * (oc + 1)], acc)
                    out3 = out_p.rearrange("(oc p) s -> p oc s", oc=4)
                    src3 = ostp[par][:].rearrange("p (oc s) -> p oc s", oc=4)
                    nc.sync.dma_start(
                        out3[:, 2 * ocp:2 * ocp + 2,
                             512 * par:512 * (par + 1)],
                        src3[:, 2 * ocp:2 * ocp + 2, :])

            emit_pre2([(0, 0), (0, 1)])
            emit_pre2([(0, 2), (0, 3)])
            emit_o(NH - 1, 2)
            emit_pre2([(1, 0), (1, 1)])
            emit_o(NH - 1, 3)
            emit_norm(NH - 1, split=True)
            # fins for s2d column half 0 (g0,g1) only need the qc0 half of
            # the last chain; g2,g3 the qc1 half
            for par, g in [(0, 0), (0, 1), (1, 0), (1, 1),
                           (0, 2), (0, 3), (1, 2), (1, 3)]:
                emit_fin(par, g, nc.vector.tensor_copy if g % 2 == 0
                         else nc.scalar.copy)
            emit_conv_s(0)
            emit_conv_s(1)

    nc.compile()
    _CACHE["nc"] = nc
    return nc


def _shard_inputs(content_feat, components, pos_emb, Wq, Wkv, Wproj, bproj,
                  Wconv, bconv):
    import ml_dtypes

    bf = ml_dtypes.bfloat16
    f = np.float32
    pos2 = np.asarray(pos_emb, dtype=f).reshape(S, C)
    wq16 = np.asarray(Wq, dtype=f).astype(bf)
    wkv16 = np.asarray(Wkv, dtype=f).astype(bf)
    wp16 = np.asarray(Wproj, dtype=f).astype(bf)
    wcT = np.ascontiguousarray(np.asarray(Wconv, dtype=f).T)
    wcs16 = np.ascontiguousarray(wcT[:C]).astype(bf)
    in_maps = []
    for core in range(N_CORES):
        b, n = core // 4, core % 4
        ctokT = np.ascontiguousarray(
            (np.asarray(content_feat[b], dtype=f).reshape(S, C) + pos2).T)
        compT = np.ascontiguousarray(
            (np.asarray(components[n, b], dtype=f).reshape(S, C) + pos2).T)
        in_maps.append({
            "comp16": compT.astype(bf),
            "ctok16": ctokT.astype(bf),
            "cf16": np.ascontiguousarray(
                np.asarray(content_feat[b], dtype=f).reshape(C, S)).astype(bf),
            "wq16": wq16,
            "wkv16": wkv16,
            "wp16": wp16,
            "wcs16": wcs16,
            "wcc16": np.ascontiguousarray(
                wcT[C:, 128 * n:128 * (n + 1)]).astype(bf),
            "gate": np.full((128, 1), 1.0 if n == 0 else 0.0, dtype=f),
        })
    return in_maps


def _run(trace=False, **inputs):
    from concourse.bass_utils import run_bass_kernel_spmd

    nc = _build()
    in_maps = _shard_inputs(**inputs)
    res = run_bass_kernel_spmd(nc, in_maps, list(range(N_CORES)), trace=trace)
    outs = [np.asarray(res.results[i]["out_p"], dtype=np.float64)
            for i in range(N_CORES)]
    out = np.stack([outs[0] + outs[1] + outs[2] + outs[3],
                    outs[4] + outs[5] + outs[6] + outs[7]], axis=0)
    for core in range(N_CORES):
        b, n = core // 4, core % 4
        out[b, 128 * n:128 * (n + 1), :] += np.asarray(
            res.results[core]["out_cf"], dtype=np.float64)
    # host-side affine constants: out += bconv[o] + ws[o]*bproj[i%512]
    # with ws[o] = sum_c Wconv[o, c<C]  (bproj enters via the conv s-half)
    Wconv = np.asarray(inputs["Wconv"], dtype=np.float64)
    bproj = np.asarray(inputs["bproj"], dtype=np.float64)
    bconv = np.asarray(inputs["bconv"], dtype=np.float64)
    ws = Wconv[:, :C].sum(axis=1)
    bias_img = bconv[:, None] + np.outer(ws, np.concatenate([bproj, bproj]))
    out += bias_img[None, :, :]
    return out.reshape(B, C, H, W).astype(np.float32), res


def kernel(**inputs):
    out, _ = _run(trace=False, **inputs)
    return out


# revision 21
# speedup vs baseline: 1.2338x; 1.0153x over previous
"""Trainium2 Bass kernel for nn_Attention_54391465836966.

Math (per batch b, component n; one (b,n) pair per core, host sums over n):
  ctok = content_feat[b].raw_reshape(S,C) + pos          # [1024, 512]
  comp_tok = components[n,b].raw_reshape(S,C) + pos
  q = ctok @ Wq ; k,v = comp_tok @ Wkv (split)
  per head h: P = exp(scale q_h k_h^T); o_h = (P @ v_h) / rowsum(P)
  s_in = concat_h(o_h) + gate*ctok                        # gate = [n==0]
  s2d  = raw_reshape(s_in @ Wproj)                        # bias via host
  out  = Wconv^T[:C] @ s2d + Wconv^T[C:] @ cf             # + bias via host

Design notes (cost-model-driven):
- The P@V (o) matmuls run fp8 DoubleRow (2 k-tiles per instruction at 0.5
  cycles/row = 4x the bf16 rate): pt = exp output in e5m2 (its 22-efold
  dynamic range covers the unnormalized exp spread -- score sigma ~2.0 ->
  +-11 efolds; e4m3's 11.7 efolds would overflow), v drained to e4m3.
  Attention-output noise from fp8 is diluted by the ctok residual and
  P-quantization partially cancels against Z (measured 1.4e-2 total).
- The q/k path stays bf16 end-to-end: fp8 anywhere before the exp adds
  ~0.17 absolute score noise -> 17% P-reweighting with only ~19 effective
  keys after softmax (score sigma 2.0) -> 3-4e-2 output error (measured).
- exp writes pt e5m2 directly with a constant -4 bias (normalization
  cancels it exactly; caps pt at exp(score-4) with overflow only past
  7.5 sigma scores, flush below -7.1 negligible).
- Z rides row 0 of the o PSUM tile (ones column first in v): the custom-DVE
  reciprocal reads partition 0 of PSUM directly (hw quirk: it ignores the
  AP's partition base), skipping the per-head z-copy entirely.  Drain +
  normalize fuse into one scalar_tensor_tensor from PSUM (v data sits at
  e=64:128 so the o data rows land at the 32-aligned partition base 64).
- The projection uses stride-2 column slices of s_in^T as the stationary
  operand, which makes its PSUM output land directly in the s->s2d
  raw-reshape layout: no PE transposes anywhere in the kernel.
- bproj/bconv are affine constants independent of the data path; their
  contribution is bconv[o] + (sum_c Wconv[o,c<C]) * bproj[i%512], a rank-1
  image the host adds during unshard.
- PSUM: score tiles tag "sc" [128,1024] bufs=3 (6 banks) shared by the
  QKV-projection accumulations (pre-phase + interleaved jobs borrow
  rotation slots); o tag [128,1024] bufs=1 (2 banks).  GPSIMD cannot touch
  PSUM, so all PSUM drains go to DVE (ACT helps only post-exp in the tail).
- ACT runs exps only (64 x [128,1024] ~= 1.04us each); QKV/conv_cf jobs are
  split into ~2048-cycle half-jobs and interleaved between score matmuls so
  the exp stream never starves.  o matmuls for head h trail into head h+1's
  first slots; the recip->broadcast->normalize chain then frees the single
  o buffer before head h+1 needs it.
"""
import sys

sys.path.insert(0, "/opt/trn_rl_repo")

import numpy as np

N_CORES = 8
B, C, H, W = 2, 512, 32, 32
S = H * W  # 1024
NH, HD = 8, 64
SCALE = HD ** -0.5
EXP_BIAS = -4.0

_CACHE = {}


def _build():
    if "nc" in _CACHE:
        return _CACHE["nc"]
    from contextlib import ExitStack

    import concourse.bacc as bacc
    import concourse.mybir as mybir
    import concourse.tile as tile

    f32 = mybir.dt.float32
    bf16 = mybir.dt.bfloat16
    fp8 = mybir.dt.float8e4
    fp8e5 = mybir.dt.float8e5
    EXP = mybir.ActivationFunctionType.Exp
    MULT = mybir.AluOpType.mult
    ADD = mybir.AluOpType.add
    BYPASS = mybir.AluOpType.bypass
    DR = mybir.MatmulPerfMode.DoubleRow

    nc = bacc.Bacc("TRN2", target_bir_lowering=False, debug=False,
                   num_devices=N_CORES)

    din = lambda n, s, dt: nc.dram_tensor(n, s, dt, kind="ExternalInput").ap()
    comp16d = din("comp16", [C, S], bf16)   # (comp_tok + pos).T
    ctok16d = din("ctok16", [C, S], bf16)   # (content_tok + pos).T
    cf16d = din("cf16", [C, S], bf16)       # content_feat[b] raw [C,S]
    wq16d = din("wq16", [C, C], bf16)       # Wq
    wkv16d = din("wkv16", [C, 2 * C], bf16)  # Wkv (cols 0:C K, C:2C V)
    wp16d = din("wp16", [C, C], bf16)       # Wproj
    wcs16d = din("wcs16", [C, C], bf16)     # Wconv.T rows :C (s half)
    wcc16d = din("wcc16", [C, 128], bf16)   # Wconv.T[C:, 128n:128(n+1)]
    gated = din("gate", [128, 1], f32)      # 1.0 on n==0 cores else 0.0
    out_p = nc.dram_tensor("out_p", [C, S], bf16, kind="ExternalOutput").ap()
    out_cf = nc.dram_tensor("out_cf", [128, S], bf16,
                            kind="ExternalOutput").ap()

    with tile.TileContext(nc) as tc, ExitStack() as ctx:
        main = ctx.enter_context(tc.tile_pool(name="main", bufs=1))

        g_sb = main.tile([128, 1], f32, tag="g")
        ebias = main.tile([128, 1], f32, tag="eb")
        nc.gpsimd.memset(ebias[:], EXP_BIAS)

        # ---- persistent SBUF tiles (one merged DMA per DRAM tensor) ----
        comp16 = main.tile([128, 4 * S], bf16, tag="cm16", name="cm16")
        ctok16 = main.tile([128, 4 * S], bf16, tag="ct16", name="ct16")
        cf16 = main.tile([128, 4 * S], bf16, tag="cf16", name="cf16")
        wq16 = main.tile([128, 4 * C], bf16, tag="wq16", name="wq16")
        wkv16 = main.tile([128, 4 * 2 * C], bf16, tag="wkv16", name="wkv16")
        wp16 = main.tile([128, 4 * C], bf16, tag="wp16", name="wp16")
        wcs16 = main.tile([128, 4 * C], bf16, tag="wcs16", name="wcs16")
        wcc16 = main.tile([128, 4 * 128], bf16, tag="wcc16", name="wcc16")

        comp_c = [comp16[:, S * k:S * (k + 1)] for k in range(4)]
        ctok_c = [ctok16[:, S * k:S * (k + 1)] for k in range(4)]
        cf_c = [cf16[:, S * k:S * (k + 1)] for k in range(4)]
        wq_c = [wq16[:, C * k:C * (k + 1)] for k in range(4)]
        wkv_c = [wkv16[:, 2 * C * k:2 * C * (k + 1)] for k in range(4)]
        wp_c = [wp16[:, C * k:C * (k + 1)] for k in range(4)]
        wcs_c = [wcs16[:, C * k:C * (k + 1)] for k in range(4)]
        wcc_c = [wcc16[:, 128 * k:128 * (k + 1)] for k in range(4)]

        kT16 = [main.tile([128, S], bf16, tag=f"kt{j}", name=f"kt{j}")
                for j in range(4)]
        qT16 = [main.tile([128, S], bf16, tag=f"qt{j}", name=f"qt{j}")
                for j in range(4)]
        # v pair tiles: [128 keys, (t2=2, h=8, e=128)] fp8e4; e=0 is the 1.0
        # ones column (Z lands on o row 0), e=64:128 the v data, 1:64 zeros.
        vp = [main.tile([128, 2 * 8 * 128], fp8, tag=f"vp{p}", name=f"vp{p}")
              for p in range(4)]
        rtb = [main.tile([128, S], bf16, tag=f"rt{j}", name=f"rt{j}")
               for j in range(4)]
        outcf_sb = main.tile([128, S], bf16, tag="ocf", name="ocf")

        # ---- DMA emission order: attention-critical first ----
        def dma_merged(dst_tile, src_ap, k, lo=0, hi=None):
            hi = k if hi is None else hi
            src3 = src_ap.rearrange("(k p) s -> p k s", k=k)
            dst3 = dst_tile[:].rearrange("p (k s) -> p k s", k=k)
            nc.sync.dma_start(dst3[:, lo:hi, :], src3[:, lo:hi, :])

        def dma_cols(dst_tile, src_ap, k, c0, c1):
            # column-group slice across all k row-chunks in one strided DMA
            src3 = src_ap.rearrange("(k p) s -> p k s", k=k)
            dst3 = dst_tile[:].rearrange("p (k s) -> p k s", k=k)
            nc.sync.dma_start(dst3[:, :, c0:c1], src3[:, :, c0:c1])

        # critical path for the first score matmul: comp + ctok + the j0
        # column groups of Wk/Wq (~2.25MB); everything else after
        dma_merged(comp16, comp16d, 4, 0, 2)
        dma_cols(wkv16, wkv16d, 4, 0, 128)        # Wk j0 cols
        dma_merged(comp16, comp16d, 4, 2, 4)
        dma_merged(ctok16, ctok16d, 4, 0, 2)
        dma_cols(wq16, wq16d, 4, 0, 128)          # Wq j0 cols
        dma_merged(ctok16, ctok16d, 4, 2, 4)
        nc.sync.dma_start(g_sb[:], gated[:])
        dma_cols(wkv16, wkv16d, 4, 512, 1024)     # V half (v jobs, early)
        dma_cols(wkv16, wkv16d, 4, 128, 512)      # Wk j1-3
        dma_cols(wq16, wq16d, 4, 128, 512)        # Wq j1-3
        dma_merged(cf16, cf16d, 4)
        dma_merged(wcc16, wcc16d, 4)
        dma_merged(wp16, wp16d, 4)
        dma_merged(wcs16, wcs16d, 4)

        # ones + zero-pad columns of the v tiles (SBUF memset = Pool)
        warm_src = main.tile([128, 128], bf16, tag="warm", name="warm")
        nc.gpsimd.memset(warm_src[:], 0.25)
        # preload the Exp activation table (1.28us) off the critical stream
        dummy_pt = main.tile([1, 8], fp8e5, tag="dpt", name="dpt")
        nc.scalar.activation(dummy_pt[0:1, :], warm_src[0:1, 0:8], EXP,
                             bias=ebias[0:1, 0:1], scale=SCALE)

        for p in range(4):
            vv = vp[p][:].rearrange("p (t h e) -> p t h e", t=2, h=8)
            nc.gpsimd.memset(vv[:, :, :, 0:1], 1.0)
            nc.gpsimd.memset(vv[:, :, :, 1:64], 0.0)

        with tc.tile_pool(name="psMain", bufs=1, space="PSUM") as ps:
            # p-state spin-up gated on the first comp chunk so the ramp is
            # still warm when the first kq matmuls run (dep-free warms would
            # finish during the DMA wait and let the clock reset)
            for _ in range(12):
                wtp = ps.tile([128, 1024], f32, tag="sc", bufs=3)
                nc.tensor.matmul(wtp[:, 0:128], warm_src[:],
                                 comp16[:, 0:128], start=True, stop=True)

            # ---- QKV projection half-jobs (bf16, ~2048 PE cycles each) ----
            kq_acc = {}

            def emit_kq_half(dst, w_c_, x_c_, j, t):
                if (id(dst), j) not in kq_acc:
                    kq_acc[(id(dst), j)] = ps.tile(
                        [128, 1024], f32, tag="sc", bufs=3, name=f"kqa{j}")
                acc = kq_acc[(id(dst), j)]
                for k in range(4):
                    nc.tensor.matmul(
                        acc[:, 512 * t:512 * (t + 1)],
                        w_c_[k][:, 128 * j:128 * (j + 1)],
                        x_c_[k][:, 512 * t:512 * (t + 1)],
                        start=(k == 0), stop=(k == 3))
                nc.vector.tensor_copy(dst[:, 512 * t:512 * (t + 1)],
                                      acc[:, 512 * t:512 * (t + 1)])
                if t == 1:
                    del kq_acc[(id(dst), j)]

            def emit_kq_q(dst, w_c_, x_c_, j, q):
                # quarter job: t = q//2, k-chunk pair = (q%2)*2
                t, k0 = q // 2, (q % 2) * 2
                if (id(dst), j) not in kq_acc:
                    kq_acc[(id(dst), j)] = ps.tile(
                        [128, 1024], f32, tag="sc", bufs=3, name=f"kqq{j}")
                acc = kq_acc[(id(dst), j)]
                for k in (k0, k0 + 1):
                    nc.tensor.matmul(
                        acc[:, 512 * t:512 * (t + 1)],
                        w_c_[k][:, 128 * j:128 * (j + 1)],
                        x_c_[k][:, 512 * t:512 * (t + 1)],
                        start=(k == 0), stop=(k == 3))
                if q % 2 == 1:
                    nc.vector.tensor_copy(dst[:, 512 * t:512 * (t + 1)],
                                          acc[:, 512 * t:512 * (t + 1)])
                if q == 3:
                    del kq_acc[(id(dst), j)]

            v_acc = {}

            def emit_v_half(p, t2):
                # token block 2p+t2; drain (strided, ->fp8e4) at t2==1.
                # pairs 0-2 accumulate in the o-tag tile (idle until the
                # first o matmul) to keep the sc rotation free for scores.
                if p not in v_acc:
                    v_acc[p] = ps.tile([128, 1024], f32,
                                       tag="o" if p < 3 else "sc",
                                       bufs=1 if p < 3 else 3,
                                       name=f"va{p}")
                acc = v_acc[p]
                tb = 2 * p + t2
                for k in range(4):
                    nc.tensor.matmul(
                        acc[:, 512 * t2:512 * (t2 + 1)],
                        comp_c[k][:, 128 * tb:128 * (tb + 1)],
                        wkv_c[k][:, C:2 * C],
                        start=(k == 0), stop=(k == 3))
                if t2 == 1:
                    del v_acc[p]
                    dst = vp[p][:].rearrange("p (t h e) -> p t h e", t=2, h=8)
                    src = acc[:].rearrange("p (t h d) -> p t h d", t=2, h=8)
                    nc.vector.tensor_copy(dst[:, :, :, 64:128],
                                          src[:, :, :, :])

            cf_acc = {}

            def emit_conv_cf_h(pc, half):
                if pc not in cf_acc:
                    cf_acc[pc] = ps.tile([128, 1024], f32, tag="sc", bufs=3,
                                         name=f"cfa{pc}")
                acc = cf_acc[pc]
                for g in (2 * half, 2 * half + 1):
                    nc.tensor.matmul(acc[:, 0:512], wcc_c[g][:, :],
                                     cf_c[g][:, 512 * pc:512 * (pc + 1)],
                                     start=(g == 0), stop=(g == 3))
                if half == 1:
                    del cf_acc[pc]
                    nc.vector.tensor_copy(
                        outcf_sb[:, 512 * pc:512 * (pc + 1)], acc[:, 0:512])
                    nc.sync.dma_start(
                        out_cf[:, 512 * pc:512 * (pc + 1)],
                        outcf_sb[:, 512 * pc:512 * (pc + 1)])

            def warm_on(src_cols, n):
                # spin the PE clock on matmuls gated by a late DMA chunk so
                # the busy-run is alive when the real matmuls become ready
                for _ in range(n):
                    wtp = ps.tile([128, 1024], f32, tag="sc", bufs=3)
                    nc.tensor.matmul(wtp[:, 0:128], warm_src[:], src_cols,
                                     start=True, stop=True)

            # pre-phase: just enough for sc(0,0): kT0 keys 0:512, qT0 full
            warm_on(comp16[:, 3 * S:3 * S + 128], 2)
            emit_kq_half(kT16[0], wkv_c, comp_c, 0, 0)
            warm_on(ctok16[:, 0:128], 2)
            emit_kq_half(qT16[0], wq_c, ctok_c, 0, 0)
            warm_on(ctok16[:, 3 * S:3 * S + 128], 2)
            emit_kq_half(qT16[0], wq_c, ctok_c, 0, 1)

            # interleaved half-jobs, keyed by the (head, kt) slot AFTER whose
            # score-matmuls they are emitted.  v pair p is needed by the o
            # matmul at slot (h,6)/(h,7)/(h+1,0)/(h+1,1); kT/qT j by (2j,0).
            ilv = {
                (0, 0): lambda: emit_kq_half(kT16[0], wkv_c, comp_c, 0, 1),
                (0, 1): lambda: emit_v_half(0, 0),
                (0, 2): lambda: emit_v_half(0, 1),
                (0, 3): lambda: emit_v_half(1, 0),
                (0, 4): lambda: emit_v_half(1, 1),
                (0, 5): lambda: emit_v_half(2, 0),
                (0, 6): lambda: emit_v_half(2, 1),
                (0, 7): lambda: emit_v_half(3, 0),
                (1, 0): lambda: emit_v_half(3, 1),
                (1, 1): lambda: emit_kq_q(kT16[1], wkv_c, comp_c, 1, 0),
                (1, 2): lambda: emit_kq_q(kT16[1], wkv_c, comp_c, 1, 1),
                (1, 3): lambda: emit_kq_q(kT16[1], wkv_c, comp_c, 1, 2),
                (1, 4): lambda: emit_kq_q(kT16[1], wkv_c, comp_c, 1, 3),
                (1, 5): lambda: emit_kq_half(qT16[1], wq_c, ctok_c, 1, 0),
                (1, 6): lambda: emit_kq_q(qT16[1], wq_c, ctok_c, 1, 2),
                (1, 7): lambda: emit_kq_q(qT16[1], wq_c, ctok_c, 1, 3),
                (2, 1): lambda: emit_kq_q(kT16[2], wkv_c, comp_c, 2, 0),
                (2, 2): lambda: emit_kq_q(kT16[2], wkv_c, comp_c, 2, 1),
                (2, 3): lambda: emit_kq_q(kT16[2], wkv_c, comp_c, 2, 2),
                (2, 4): lambda: emit_kq_q(kT16[2], wkv_c, comp_c, 2, 3),
                (2, 5): lambda: emit_kq_q(qT16[2], wq_c, ctok_c, 2, 0),
                (2, 6): lambda: emit_kq_q(qT16[2], wq_c, ctok_c, 2, 1),
                (2, 7): lambda: emit_kq_q(qT16[2], wq_c, ctok_c, 2, 2),
                (3, 1): lambda: emit_kq_q(qT16[2], wq_c, ctok_c, 2, 3),
                (3, 2): lambda: emit_kq_q(kT16[3], wkv_c, comp_c, 3, 0),
                (3, 3): lambda: emit_kq_q(kT16[3], wkv_c, comp_c, 3, 1),
                (3, 4): lambda: emit_kq_q(kT16[3], wkv_c, comp_c, 3, 2),
                (3, 5): lambda: emit_kq_q(kT16[3], wkv_c, comp_c, 3, 3),
                (3, 6): lambda: emit_kq_q(qT16[3], wq_c, ctok_c, 3, 0),
                (3, 7): lambda: emit_kq_q(qT16[3], wq_c, ctok_c, 3, 1),
                (4, 1): lambda: emit_kq_q(qT16[3], wq_c, ctok_c, 3, 2),
                (4, 2): lambda: emit_kq_q(qT16[3], wq_c, ctok_c, 3, 3),
                (5, 1): lambda: emit_conv_cf_h(0, 0),
                (5, 2): lambda: emit_conv_cf_h(0, 1),
                (5, 4): lambda: emit_conv_cf_h(1, 0),
                (5, 5): lambda: emit_conv_cf_h(1, 1),
            }

            # ---- attention ----
            pt_pool = {}      # (h, p) -> pt pair tile [128, 2048] e5m2
            o_tiles = {}      # h -> o psum tile

            def emit_sc(h, kt):
                jq, row = h // 2, 64 * (h % 2)
                sc = ps.tile([128, S], f32, tag="sc", bufs=3)
                for qc in range(2):
                    nc.tensor.matmul(
                        sc[:, 512 * qc:512 * (qc + 1)],
                        kT16[jq][row:row + 64, 128 * kt:128 * (kt + 1)],
                        qT16[jq][row:row + 64, 512 * qc:512 * (qc + 1)],
                        start=True, stop=True)
                if kt % 2 == 0:
                    pt_pool[(h, kt // 2)] = main.tile(
                        [128, 2048], fp8e5, tag="pt", bufs=6,
                        name=f"pt{h}_{kt // 2}")
                pt = pt_pool[(h, kt // 2)]
                nc.scalar.activation(pt[:, 1024 * (kt % 2):1024 * (kt % 2 + 1)],
                                     sc[:], EXP, bias=ebias[:, 0:1],
                                     scale=SCALE)

            def emit_o(h, p):
                # o[0,:] = Z, o[64:128,:] = P@v_h  (DoubleRow over kt pair)
                if h not in o_tiles:
                    o_tiles[h] = ps.tile([128, S], f32, tag="o", bufs=1,
                                         name=f"o{h}")
                o_ps = o_tiles[h]
                vv = vp[p][:].rearrange("p (t h e) -> p t h e", t=2, h=8)
                ptv = pt_pool.pop((h, p))[:].rearrange("p (t q) -> p t q", t=2)
                for qc in range(2):
                    nc.tensor.matmul(
                        o_ps[:, 512 * qc:512 * (qc + 1)],
                        vv[:, :, h:h + 1, :],
                        ptv[:, :, 512 * qc:512 * (qc + 1)],
                        start=(p == 0), stop=(p == 3), perf_mode=DR)

            def emit_norm(h, split=False):
                jq, row = h // 2, 64 * (h % 2)
                o_ps = o_tiles.pop(h)
                if split:
                    # qc-halved chain, recips emitted first so half 1's
                    # recip overlaps half 0's Pool broadcast
                    zis, zbs = [], []
                    for qc in range(2):
                        zis.append(main.tile([1, 512], f32, tag="zis",
                                             bufs=2, name=f"zis{h}{qc}"))
                        zbs.append(main.tile([64, 512], f32, tag="zbs",
                                             bufs=2, name=f"zbs{h}{qc}"))
                    for qc in range(2):
                        nc.vector.reciprocal_approx_fast(
                            zis[qc][0:1, :],
                            o_ps[0:1, 512 * qc:512 * (qc + 1)])
                    for qc in range(2):
                        nc.gpsimd.partition_broadcast(zbs[qc][0:64, :],
                                                      zis[qc][0:1, :])
                    for qc in range(2):
                        nc.vector.scalar_tensor_tensor(
                            rtb[jq][row:row + 64, 512 * qc:512 * (qc + 1)],
                            o_ps[64:128, 512 * qc:512 * (qc + 1)], 1.0,
                            zbs[qc][0:64, :], BYPASS, MULT)
                        if h % 2 == 1:
                            nc.vector.scalar_tensor_tensor(
                                rtb[jq][:, 512 * qc:512 * (qc + 1)],
                                ctok_c[jq][:, 512 * qc:512 * (qc + 1)],
                                g_sb[:, 0:1],
                                rtb[jq][:, 512 * qc:512 * (qc + 1)],
                                MULT, ADD)
                    return
                else:
                    zi = main.tile([1, S], f32, tag="zi", bufs=2,
                                   name=f"zi{h}")
                    zbc = main.tile([64, S], f32, tag="zb", bufs=2,
                                    name=f"zb{h}")
                    nc.vector.reciprocal_approx_fast(zi[0:1, :], o_ps[0:1, :])
                    nc.gpsimd.partition_broadcast(zbc[0:64, :], zi[0:1, :])
                    nc.vector.scalar_tensor_tensor(
                        rtb[jq][row:row + 64, :], o_ps[64:128, :], 1.0,
                        zbc[0:64, :], BYPASS, MULT)
                if h % 2 == 1:  # pair complete: s_in^T = rtb + gate*ctokT
                    nc.vector.scalar_tensor_tensor(
                        rtb[jq][:], ctok_c[jq][:], g_sb[:, 0:1], rtb[jq][:],
                        MULT, ADD)

            # flat emission; o(h) trails into head h+1 per the o-bank cycle
            for h in range(NH):
                for kt in range(8):
                    emit_sc(h, kt)
                    if h > 0:
                        if kt == 0:
                            emit_o(h - 1, 2)
                        elif kt == 1:
                            emit_o(h - 1, 3)
                        elif kt == 2:
                            emit_norm(h - 1)
                    if (h, kt) in ilv:
                        ilv[(h, kt)]()
                    if kt == 6:
                        emit_o(h, 0)
                    elif kt == 7:
                        emit_o(h, 1)
            # ---- tail (same pool: proj/conv accumulators ride the
            # sc/o tags as [128,512] halves; proj j0-2 partials fill the
            # end of the exp stream before the last o matmuls) ----
            rtb_v = [rtb[j][:].rearrange("p (g i two) -> p g two i",
                                         g=4, two=2) for j in range(4)]
            s2d_sb = [[main.tile([128, 512], bf16, tag=f"s2d{par}{g}",
                                 name=f"s2d{par}{g}") for g in range(4)]
                      for par in range(2)]
            ostp = [main.tile([128, 4 * 512], bf16, tag=f"ostp{par}",
                              name=f"ostp{par}") for par in range(2)]
            pj = {}

            def emit_pre2(pairs):
                tl_ = ps.tile([128, 1024], f32, tag="sc", bufs=3,
                              name=f"pj{pairs[0][0]}{pairs[0][1]}")
                for i, (par, g) in enumerate(pairs):
                    acc = tl_[:, 512 * i:512 * (i + 1)]
                    pj[(par, g)] = acc
                    for j in range(3):
                        nc.tensor.matmul(acc, rtb_v[j][:, g, par, :],
                                         wp_c[j][:, :], start=(j == 0),
                                         stop=False)

            def emit_fin(par, g, eng):
                acc = pj.pop((par, g), None)
                if acc is None:
                    # not pre-started: full 4-chain into the o-tag tile half
                    if "otl" not in pj:
                        pj["otl"] = ps.tile([128, 1024], f32, tag="o",
                                            bufs=1, name="pjo")
                    acc = pj["otl"][:, 512 * (g % 2):512 * (g % 2 + 1)]
                    for j in range(4):
                        nc.tensor.matmul(acc, rtb_v[j][:, g, par, :],
                                         wp_c[j][:, :], start=(j == 0),
                                         stop=(j == 3))
                else:
                    nc.tensor.matmul(acc, rtb_v[3][:, g, par, :],
                                     wp_c[3][:, :], start=False, stop=True)
                eng(s2d_sb[par][g][:], acc)

            def emit_conv_s(par):
                for ocp in range(2):
                    tl_ = ps.tile([128, 1024], f32, tag="sc", bufs=3,
                                  name=f"cv{par}{ocp}")
                    for i in range(2):
                        oc = 2 * ocp + i
                        acc = tl_[:, 512 * i:512 * (i + 1)]
                        for g in range(4):
                            nc.tensor.matmul(
                                acc, wcs_c[g][:, 128 * oc:128 * (oc + 1)],
                                s2d_sb[par][g][:],
                                start=(g == 0), stop=(g == 3))
                        eng = (nc.vector.tensor_copy if oc % 2 == 0
                               else nc.scalar.copy)
                        eng(ostp[par][:, 512 * oc:512 * (oc + 1)], acc)
                    out3 = out_p.rearrange("(oc p) s -> p oc s", oc=4)
                    src3 = ostp[par][:].rearrange("p (oc s) -> p oc s", oc=4)
                    nc.sync.dma_start(
                        out3[:, 2 * ocp:2 * ocp + 2,
                             512 * par:512 * (par + 1)],
                        src3[:, 2 * ocp:2 * ocp + 2, :])

            emit_pre2([(0, 0), (0, 1)])
            emit_pre2([(0, 2), (0, 3)])
            emit_o(NH - 1, 2)
            emit_pre2([(1, 0), (1, 1)])
            emit_o(NH - 1, 3)
            emit_norm(NH - 1, split=True)
            # fins for s2d column half 0 (g0,g1) only need the qc0 half of
            # the last chain; g2,g3 the qc1 half
            for par, g in [(0, 0), (0, 1), (1, 0), (1, 1),
                           (0, 2), (0, 3), (1, 2), (1, 3)]:
                emit_fin(par, g, nc.vector.tensor_copy if g % 2 == 0
                         else nc.scalar.copy)
            emit_conv_s(0)
            emit_conv_s(1)

    nc.compile()
    _CACHE["nc"] = nc
    return nc


def _shard_inputs(content_feat, components, pos_emb, Wq, Wkv, Wproj, bproj,
                  Wconv, bconv):
    import ml_dtypes

    bf = ml_dtypes.bfloat16
    f = np.float32
    pos2 = np.asarray(pos_emb, dtype=f).reshape(S, C)
    wq16 = np.asarray(Wq, dtype=f).astype(bf)
    wkv16 = np.asarray(Wkv, dtype=f).astype(bf)
    wp16 = np.asarray(Wproj, dtype=f).astype(bf)
    wcT = np.ascontiguousarray(np.asarray(Wconv, dtype=f).T)
    wcs16 = np.ascontiguousarray(wcT[:C]).astype(bf)
    in_maps = []
    for core in range(N_CORES):
        b, n = core // 4, core % 4
        ctokT = np.ascontiguousarray(
            (np.asarray(content_feat[b], dtype=f).reshape(S, C) + pos2).T)
        compT = np.ascontiguousarray(
            (np.asarray(components[n, b], dtype=f).reshape(S, C) + pos2).T)
        in_maps.append({
            "comp16": compT.astype(bf),
            "ctok16": ctokT.astype(bf),
            "cf16": np.ascontiguousarray(
                np.asarray(content_feat[b], dtype=f).reshape(C, S)).astype(bf),
            "wq16": wq16,
            "wkv16": wkv16,
            "wp16": wp16,
            "wcs16": wcs16,
            "wcc16": np.ascontiguousarray(
                wcT[C:, 128 * n:128 * (n + 1)]).astype(bf),
            "gate": np.full((128, 1), 1.0 if n == 0 else 0.0, dtype=f),
        })
    return in_maps


def _run(trace=False, **inputs):
    from concourse.bass_utils import run_bass_kernel_spmd

    nc = _build()
    in_maps = _shard_inputs(**inputs)
    res = run_bass_kernel_spmd(nc, in_maps, list(range(N_CORES)), trace=trace)
    outs = [np.asarray(res.results[i]["out_p"], dtype=np.float64)
            for i in range(N_CORES)]
    out = np.stack([outs[0] + outs[1] + outs[2] + outs[3],
                    outs[4] + outs[5] + outs[6] + outs[7]], axis=0)
    for core in range(N_CORES):
        b, n = core // 4, core % 4
        out[b, 128 * n:128 * (n + 1), :] += np.asarray(
            res.results[core]["out_cf"], dtype=np.float64)
    # host-side affine constants: out += bconv[o] + ws[o]*bproj[i%512]
    # with ws[o] = sum_c Wconv[o, c<C]  (bproj enters via the conv s-half)
    Wconv = np.asarray(inputs["Wconv"], dtype=np.float64)
    bproj = np.asarray(inputs["bproj"], dtype=np.float64)
    bconv = np.asarray(inputs["bconv"], dtype=np.float64)
    ws = Wconv[:, :C].sum(axis=1)
    bias_img = bconv[:, None] + np.outer(ws, np.concatenate([bproj, bproj]))
    out += bias_img[None, :, :]
    return out.reshape(B, C, H, W).astype(np.float32), res


def kernel(**inputs):
    out, _ = _run(trace=False, **inputs)
    return out


# revision 22
# speedup vs baseline: 1.2523x; 1.0150x over previous
"""Trainium2 Bass kernel for nn_Attention_54391465836966.

Math (per batch b, component n; one (b,n) pair per core, host sums over n):
  ctok = content_feat[b].raw_reshape(S,C) + pos          # [1024, 512]
  comp_tok = components[n,b].raw_reshape(S,C) + pos
  q = ctok @ Wq ; k,v = comp_tok @ Wkv (split)
  per head h: P = exp(scale q_h k_h^T); o_h = (P @ v_h) / rowsum(P)
  s_in = concat_h(o_h) + gate*ctok                        # gate = [n==0]
  s2d  = raw_reshape(s_in @ Wproj)                        # bias via host
  out  = Wconv^T[:C] @ s2d + Wconv^T[C:] @ cf             # + bias via host

Design notes (cost-model-driven):
- The P@V (o) matmuls run fp8 DoubleRow (2 k-tiles per instruction at 0.5
  cycles/row = 4x the bf16 rate): pt = exp output in e5m2 (its 22-efold
  dynamic range covers the unnormalized exp spread -- score sigma ~2.0 ->
  +-11 efolds; e4m3's 11.7 efolds would overflow), v drained to e4m3.
  Attention-output noise from fp8 is diluted by the ctok residual and
  P-quantization partially cancels against Z (measured 1.4e-2 total).
- The q/k path stays bf16 end-to-end: fp8 anywhere before the exp adds
  ~0.17 absolute score noise -> 17% P-reweighting with only ~19 effective
  keys after softmax (score sigma 2.0) -> 3-4e-2 output error (measured).
- exp writes pt e5m2 directly with a constant -4 bias (normalization
  cancels it exactly; caps pt at exp(score-4) with overflow only past
  7.5 sigma scores, flush below -7.1 negligible).
- Z rides row 0 of the o PSUM tile (ones column first in v): the custom-DVE
  reciprocal reads partition 0 of PSUM directly (hw quirk: it ignores the
  AP's partition base), skipping the per-head z-copy entirely.  Drain +
  normalize fuse into one scalar_tensor_tensor from PSUM (v data sits at
  e=64:128 so the o data rows land at the 32-aligned partition base 64).
- The projection uses stride-2 column slices of s_in^T as the stationary
  operand, which makes its PSUM output land directly in the s->s2d
  raw-reshape layout: no PE transposes anywhere in the kernel.
- bproj/bconv are affine constants independent of the data path; their
  contribution is bconv[o] + (sum_c Wconv[o,c<C]) * bproj[i%512], a rank-1
  image the host adds during unshard.
- PSUM: score tiles tag "sc" [128,1024] bufs=3 (6 banks) shared by the
  QKV-projection accumulations (pre-phase + interleaved jobs borrow
  rotation slots); o tag [128,1024] bufs=1 (2 banks).  GPSIMD cannot touch
  PSUM, so all PSUM drains go to DVE (ACT helps only post-exp in the tail).
- ACT runs exps only (64 x [128,1024] ~= 1.04us each); QKV/conv_cf jobs are
  split into ~2048-cycle half-jobs and interleaved between score matmuls so
  the exp stream never starves.  o matmuls for head h trail into head h+1's
  first slots; the recip->broadcast->normalize chain then frees the single
  o buffer before head h+1 needs it.
"""
import sys

sys.path.insert(0, "/opt/trn_rl_repo")

import numpy as np

N_CORES = 8
B, C, H, W = 2, 512, 32, 32
S = H * W  # 1024
NH, HD = 8, 64
SCALE = HD ** -0.5
EXP_BIAS = -4.0

_CACHE = {}


def _build():
    if "nc" in _CACHE:
        return _CACHE["nc"]
    from contextlib import ExitStack

    import concourse.bacc as bacc
    import concourse.mybir as mybir
    import concourse.tile as tile

    f32 = mybir.dt.float32
    bf16 = mybir.dt.bfloat16
    fp8 = mybir.dt.float8e4
    fp8e5 = mybir.dt.float8e5
    EXP = mybir.ActivationFunctionType.Exp
    MULT = mybir.AluOpType.mult
    ADD = mybir.AluOpType.add
    BYPASS = mybir.AluOpType.bypass
    DR = mybir.MatmulPerfMode.DoubleRow

    nc = bacc.Bacc("TRN2", target_bir_lowering=False, debug=False,
                   num_devices=N_CORES)

    din = lambda n, s, dt: nc.dram_tensor(n, s, dt, kind="ExternalInput").ap()
    comp16d = din("comp16", [C, S], bf16)   # (comp_tok + pos).T
    ctok16d = din("ctok16", [C, S], bf16)   # (content_tok + pos).T
    cf16d = din("cf16", [C, S], bf16)       # content_feat[b] raw [C,S]
    wq16d = din("wq16", [C, C], bf16)       # Wq
    wkv16d = din("wkv16", [C, 2 * C], bf16)  # Wkv (cols 0:C K, C:2C V)
    wp16d = din("wp16", [C, C], bf16)       # Wproj
    wcs16d = din("wcs16", [C, C], bf16)     # Wconv.T rows :C (s half)
    wcc16d = din("wcc16", [C, 128], bf16)   # Wconv.T[C:, 128n:128(n+1)]
    gated = din("gate", [128, 1], f32)      # 1.0 on n==0 cores else 0.0
    out_p = nc.dram_tensor("out_p", [C, S], bf16, kind="ExternalOutput").ap()
    out_cf = nc.dram_tensor("out_cf", [128, S], bf16,
                            kind="ExternalOutput").ap()

    with tile.TileContext(nc) as tc, ExitStack() as ctx:
        main = ctx.enter_context(tc.tile_pool(name="main", bufs=1))

        g_sb = main.tile([128, 1], f32, tag="g")
        ebias = main.tile([128, 1], f32, tag="eb")
        nc.gpsimd.memset(ebias[:], EXP_BIAS)

        # ---- persistent SBUF tiles (one merged DMA per DRAM tensor) ----
        comp16 = main.tile([128, 4 * S], bf16, tag="cm16", name="cm16")
        ctok16 = main.tile([128, 4 * S], bf16, tag="ct16", name="ct16")
        cf16 = main.tile([128, 4 * S], bf16, tag="cf16", name="cf16")
        wq16 = main.tile([128, 4 * C], bf16, tag="wq16", name="wq16")
        wkv16 = main.tile([128, 4 * 2 * C], bf16, tag="wkv16", name="wkv16")
        wp16 = main.tile([128, 4 * C], bf16, tag="wp16", name="wp16")
        wcs16 = main.tile([128, 4 * C], bf16, tag="wcs16", name="wcs16")
        wcc16 = main.tile([128, 4 * 128], bf16, tag="wcc16", name="wcc16")

        comp_c = [comp16[:, S * k:S * (k + 1)] for k in range(4)]
        ctok_c = [ctok16[:, S * k:S * (k + 1)] for k in range(4)]
        cf_c = [cf16[:, S * k:S * (k + 1)] for k in range(4)]
        wq_c = [wq16[:, C * k:C * (k + 1)] for k in range(4)]
        wkv_c = [wkv16[:, 2 * C * k:2 * C * (k + 1)] for k in range(4)]
        wp_c = [wp16[:, C * k:C * (k + 1)] for k in range(4)]
        wcs_c = [wcs16[:, C * k:C * (k + 1)] for k in range(4)]
        wcc_c = [wcc16[:, 128 * k:128 * (k + 1)] for k in range(4)]

        kT16 = [main.tile([128, S], bf16, tag=f"kt{j}", name=f"kt{j}")
                for j in range(4)]
        qT16 = [main.tile([128, S], bf16, tag=f"qt{j}", name=f"qt{j}")
                for j in range(4)]
        # v pair tiles: [128 keys, (t2=2, h=8, e=128)] fp8e4; e=0 is the 1.0
        # ones column (Z lands on o row 0), e=64:128 the v data, 1:64 zeros.
        vp = [main.tile([128, 2 * 8 * 128], fp8, tag=f"vp{p}", name=f"vp{p}")
              for p in range(4)]
        rtb = [main.tile([128, S], bf16, tag=f"rt{j}", name=f"rt{j}")
               for j in range(4)]
        outcf_sb = main.tile([128, S], bf16, tag="ocf", name="ocf")

        # ---- DMA emission order: attention-critical first ----
        def dma_merged(dst_tile, src_ap, k, lo=0, hi=None):
            hi = k if hi is None else hi
            src3 = src_ap.rearrange("(k p) s -> p k s", k=k)
            dst3 = dst_tile[:].rearrange("p (k s) -> p k s", k=k)
            nc.sync.dma_start(dst3[:, lo:hi, :], src3[:, lo:hi, :])

        def dma_cols(dst_tile, src_ap, k, c0, c1):
            # column-group slice across all k row-chunks in one strided DMA
            src3 = src_ap.rearrange("(k p) s -> p k s", k=k)
            dst3 = dst_tile[:].rearrange("p (k s) -> p k s", k=k)
            nc.sync.dma_start(dst3[:, :, c0:c1], src3[:, :, c0:c1])

        # critical path for the first score matmul: comp + ctok + the j0
        # column groups of Wk/Wq (~2.25MB); everything else after
        dma_merged(comp16, comp16d, 4, 0, 2)
        dma_cols(wkv16, wkv16d, 4, 0, 128)        # Wk j0 cols
        dma_merged(comp16, comp16d, 4, 2, 4)
        dma_merged(ctok16, ctok16d, 4, 0, 2)
        dma_cols(wq16, wq16d, 4, 0, 128)          # Wq j0 cols
        dma_merged(ctok16, ctok16d, 4, 2, 4)
        nc.sync.dma_start(g_sb[:], gated[:])
        dma_cols(wkv16, wkv16d, 4, 512, 1024)     # V half (v jobs, early)
        dma_cols(wkv16, wkv16d, 4, 128, 512)      # Wk j1-3
        dma_cols(wq16, wq16d, 4, 128, 512)        # Wq j1-3
        dma_merged(cf16, cf16d, 4)
        dma_merged(wcc16, wcc16d, 4)
        dma_merged(wp16, wp16d, 4)
        dma_merged(wcs16, wcs16d, 4)

        # ones + zero-pad columns of the v tiles (SBUF memset = Pool)
        warm_src = main.tile([128, 128], bf16, tag="warm", name="warm")
        nc.gpsimd.memset(warm_src[:], 0.25)
        # preload the Exp activation table (1.28us) off the critical stream
        dummy_pt = main.tile([1, 8], fp8e5, tag="dpt", name="dpt")
        nc.scalar.activation(dummy_pt[0:1, :], warm_src[0:1, 0:8], EXP,
                             bias=ebias[0:1, 0:1], scale=SCALE)

        for p in range(4):
            vv = vp[p][:].rearrange("p (t h e) -> p t h e", t=2, h=8)
            nc.gpsimd.memset(vv[:, :, :, 0:1], 1.0)
            nc.gpsimd.memset(vv[:, :, :, 1:64], 0.0)

        with tc.tile_pool(name="psMain", bufs=1, space="PSUM") as ps:
            # p-state spin-up gated on the first comp chunk so the ramp is
            # still warm when the first kq matmuls run (dep-free warms would
            # finish during the DMA wait and let the clock reset)
            for _ in range(12):
                wtp = ps.tile([128, 1024], f32, tag="sc", bufs=3)
                nc.tensor.matmul(wtp[:, 0:128], warm_src[:],
                                 comp16[:, 0:128], start=True, stop=True)

            # ---- QKV projection half-jobs (bf16, ~2048 PE cycles each) ----
            kq_acc = {}

            def emit_kq_half(dst, w_c_, x_c_, j, t):
                if (id(dst), j) not in kq_acc:
                    kq_acc[(id(dst), j)] = ps.tile(
                        [128, 1024], f32, tag="sc", bufs=3, name=f"kqa{j}")
                acc = kq_acc[(id(dst), j)]
                for k in range(4):
                    nc.tensor.matmul(
                        acc[:, 512 * t:512 * (t + 1)],
                        w_c_[k][:, 128 * j:128 * (j + 1)],
                        x_c_[k][:, 512 * t:512 * (t + 1)],
                        start=(k == 0), stop=(k == 3))
                nc.vector.tensor_copy(dst[:, 512 * t:512 * (t + 1)],
                                      acc[:, 512 * t:512 * (t + 1)])
                if t == 1:
                    del kq_acc[(id(dst), j)]

            def emit_kq_q(dst, w_c_, x_c_, j, q):
                # quarter job: t = q//2, k-chunk pair = (q%2)*2
                t, k0 = q // 2, (q % 2) * 2
                if (id(dst), j) not in kq_acc:
                    kq_acc[(id(dst), j)] = ps.tile(
                        [128, 1024], f32, tag="sc", bufs=3, name=f"kqq{j}")
                acc = kq_acc[(id(dst), j)]
                for k in (k0, k0 + 1):
                    nc.tensor.matmul(
                        acc[:, 512 * t:512 * (t + 1)],
                        w_c_[k][:, 128 * j:128 * (j + 1)],
                        x_c_[k][:, 512 * t:512 * (t + 1)],
                        start=(k == 0), stop=(k == 3))
                if q % 2 == 1:
                    nc.vector.tensor_copy(dst[:, 512 * t:512 * (t + 1)],
                                          acc[:, 512 * t:512 * (t + 1)])
                if q == 3:
                    del kq_acc[(id(dst), j)]

            v_acc = {}

            def emit_v_half(p, t2):
                # token block 2p+t2; drain (strided, ->fp8e4) at t2==1.
                # pairs 0-2 accumulate in the o-tag tile (idle until the
                # first o matmul) to keep the sc rotation free for scores.
                if p not in v_acc:
                    v_acc[p] = ps.tile([128, 1024], f32,
                                       tag="o" if p < 3 else "sc",
                                       bufs=1 if p < 3 else 3,
                                       name=f"va{p}")
                acc = v_acc[p]
                tb = 2 * p + t2
                for k in range(4):
                    nc.tensor.matmul(
                        acc[:, 512 * t2:512 * (t2 + 1)],
                        comp_c[k][:, 128 * tb:128 * (tb + 1)],
                        wkv_c[k][:, C:2 * C],
                        start=(k == 0), stop=(k == 3))
                if t2 == 1:
                    del v_acc[p]
                    dst = vp[p][:].rearrange("p (t h e) -> p t h e", t=2, h=8)
                    src = acc[:].rearrange("p (t h d) -> p t h d", t=2, h=8)
                    nc.vector.tensor_copy(dst[:, :, :, 64:128],
                                          src[:, :, :, :])

            cf_acc = {}

            def emit_conv_cf_h(pc, half):
                if pc not in cf_acc:
                    cf_acc[pc] = ps.tile([128, 1024], f32, tag="sc", bufs=3,
                                         name=f"cfa{pc}")
                acc = cf_acc[pc]
                for g in (2 * half, 2 * half + 1):
                    nc.tensor.matmul(acc[:, 0:512], wcc_c[g][:, :],
                                     cf_c[g][:, 512 * pc:512 * (pc + 1)],
                                     start=(g == 0), stop=(g == 3))
                if half == 1:
                    del cf_acc[pc]
                    nc.vector.tensor_copy(
                        outcf_sb[:, 512 * pc:512 * (pc + 1)], acc[:, 0:512])
                    nc.sync.dma_start(
                        out_cf[:, 512 * pc:512 * (pc + 1)],
                        outcf_sb[:, 512 * pc:512 * (pc + 1)])

            def warm_on(src_cols, n):
                # spin the PE clock on matmuls gated by a late DMA chunk so
                # the busy-run is alive when the real matmuls become ready
                for _ in range(n):
                    wtp = ps.tile([128, 1024], f32, tag="sc", bufs=3)
                    nc.tensor.matmul(wtp[:, 0:128], warm_src[:], src_cols,
                                     start=True, stop=True)

            # pre-phase: just enough for sc(0,0): kT0 keys 0:512, qT0 full
            warm_on(comp16[:, 3 * S:3 * S + 128], 2)
            emit_kq_q(kT16[0], wkv_c, comp_c, 0, 0)
            emit_kq_q(kT16[0], wkv_c, comp_c, 0, 1)
            warm_on(ctok16[:, 0:128], 2)
            emit_kq_q(qT16[0], wq_c, ctok_c, 0, 0)
            emit_kq_q(qT16[0], wq_c, ctok_c, 0, 2)
            emit_kq_q(qT16[0], wq_c, ctok_c, 0, 1)
            emit_kq_q(qT16[0], wq_c, ctok_c, 0, 3)

            # interleaved half-jobs, keyed by the (head, kt) slot AFTER whose
            # score-matmuls they are emitted.  v pair p is needed by the o
            # matmul at slot (h,6)/(h,7)/(h+1,0)/(h+1,1); kT/qT j by (2j,0).
            ilv = {
                (0, 0): lambda: (emit_kq_q(kT16[0], wkv_c, comp_c, 0, 2),
                                 emit_kq_q(kT16[0], wkv_c, comp_c, 0, 3)),
                (0, 1): lambda: emit_v_half(0, 0),
                (0, 2): lambda: emit_v_half(0, 1),
                (0, 3): lambda: emit_v_half(1, 0),
                (0, 4): lambda: emit_v_half(1, 1),
                (0, 5): lambda: emit_v_half(2, 0),
                (0, 6): lambda: emit_v_half(2, 1),
                (0, 7): lambda: emit_v_half(3, 0),
                (1, 0): lambda: emit_v_half(3, 1),
                (1, 1): lambda: emit_kq_q(kT16[1], wkv_c, comp_c, 1, 0),
                (1, 2): lambda: emit_kq_q(kT16[1], wkv_c, comp_c, 1, 1),
                (1, 3): lambda: emit_kq_q(kT16[1], wkv_c, comp_c, 1, 2),
                (1, 4): lambda: emit_kq_q(kT16[1], wkv_c, comp_c, 1, 3),
                (1, 5): lambda: emit_kq_half(qT16[1], wq_c, ctok_c, 1, 0),
                (1, 6): lambda: emit_kq_q(qT16[1], wq_c, ctok_c, 1, 2),
                (1, 7): lambda: emit_kq_q(qT16[1], wq_c, ctok_c, 1, 3),
                (2, 1): lambda: emit_kq_q(kT16[2], wkv_c, comp_c, 2, 0),
                (2, 2): lambda: emit_kq_q(kT16[2], wkv_c, comp_c, 2, 1),
                (2, 3): lambda: emit_kq_q(kT16[2], wkv_c, comp_c, 2, 2),
                (2, 4): lambda: emit_kq_q(kT16[2], wkv_c, comp_c, 2, 3),
                (2, 5): lambda: emit_kq_q(qT16[2], wq_c, ctok_c, 2, 0),
                (2, 6): lambda: emit_kq_q(qT16[2], wq_c, ctok_c, 2, 1),
                (2, 7): lambda: emit_kq_q(qT16[2], wq_c, ctok_c, 2, 2),
                (3, 1): lambda: emit_kq_q(qT16[2], wq_c, ctok_c, 2, 3),
                (3, 2): lambda: emit_kq_q(kT16[3], wkv_c, comp_c, 3, 0),
                (3, 3): lambda: emit_kq_q(kT16[3], wkv_c, comp_c, 3, 1),
                (3, 4): lambda: emit_kq_q(kT16[3], wkv_c, comp_c, 3, 2),
                (3, 5): lambda: emit_kq_q(kT16[3], wkv_c, comp_c, 3, 3),
                (3, 6): lambda: emit_kq_q(qT16[3], wq_c, ctok_c, 3, 0),
                (3, 7): lambda: emit_kq_q(qT16[3], wq_c, ctok_c, 3, 1),
                (4, 1): lambda: emit_kq_q(qT16[3], wq_c, ctok_c, 3, 2),
                (4, 2): lambda: emit_kq_q(qT16[3], wq_c, ctok_c, 3, 3),
                (5, 1): lambda: emit_conv_cf_h(0, 0),
                (5, 2): lambda: emit_conv_cf_h(0, 1),
                (5, 4): lambda: emit_conv_cf_h(1, 0),
                (5, 5): lambda: emit_conv_cf_h(1, 1),
            }

            # ---- attention ----
            pt_pool = {}      # (h, p) -> pt pair tile [128, 2048] e5m2
            o_tiles = {}      # h -> o psum tile

            def emit_sc(h, kt):
                jq, row = h // 2, 64 * (h % 2)
                sc = ps.tile([128, S], f32, tag="sc", bufs=3)
                for qc in range(2):
                    nc.tensor.matmul(
                        sc[:, 512 * qc:512 * (qc + 1)],
                        kT16[jq][row:row + 64, 128 * kt:128 * (kt + 1)],
                        qT16[jq][row:row + 64, 512 * qc:512 * (qc + 1)],
                        start=True, stop=True)
                if kt % 2 == 0:
                    pt_pool[(h, kt // 2)] = main.tile(
                        [128, 2048], fp8e5, tag="pt", bufs=6,
                        name=f"pt{h}_{kt // 2}")
                pt = pt_pool[(h, kt // 2)]
                nc.scalar.activation(pt[:, 1024 * (kt % 2):1024 * (kt % 2 + 1)],
                                     sc[:], EXP, bias=ebias[:, 0:1],
                                     scale=SCALE)

            def emit_o(h, p):
                # o[0,:] = Z, o[64:128,:] = P@v_h  (DoubleRow over kt pair)
                if h not in o_tiles:
                    o_tiles[h] = ps.tile([128, S], f32, tag="o", bufs=1,
                                         name=f"o{h}")
                o_ps = o_tiles[h]
                vv = vp[p][:].rearrange("p (t h e) -> p t h e", t=2, h=8)
                ptv = pt_pool.pop((h, p))[:].rearrange("p (t q) -> p t q", t=2)
                for qc in range(2):
                    nc.tensor.matmul(
                        o_ps[:, 512 * qc:512 * (qc + 1)],
                        vv[:, :, h:h + 1, :],
                        ptv[:, :, 512 * qc:512 * (qc + 1)],
                        start=(p == 0), stop=(p == 3), perf_mode=DR)

            def emit_norm(h, split=False):
                jq, row = h // 2, 64 * (h % 2)
                o_ps = o_tiles.pop(h)
                if split:
                    # qc-halved chain, recips emitted first so half 1's
                    # recip overlaps half 0's Pool broadcast
                    zis, zbs = [], []
                    for qc in range(2):
                        zis.append(main.tile([1, 512], f32, tag="zis",
                                             bufs=2, name=f"zis{h}{qc}"))
                        zbs.append(main.tile([64, 512], f32, tag="zbs",
                                             bufs=2, name=f"zbs{h}{qc}"))
                    for qc in range(2):
                        nc.vector.reciprocal_approx_fast(
                            zis[qc][0:1, :],
                            o_ps[0:1, 512 * qc:512 * (qc + 1)])
                    for qc in range(2):
                        nc.gpsimd.partition_broadcast(zbs[qc][0:64, :],
                                                      zis[qc][0:1, :])
                    for qc in range(2):
                        nc.vector.scalar_tensor_tensor(
                            rtb[jq][row:row + 64, 512 * qc:512 * (qc + 1)],
                            o_ps[64:128, 512 * qc:512 * (qc + 1)], 1.0,
                            zbs[qc][0:64, :], BYPASS, MULT)
                        if h % 2 == 1:
                            nc.vector.scalar_tensor_tensor(
                                rtb[jq][:, 512 * qc:512 * (qc + 1)],
                                ctok_c[jq][:, 512 * qc:512 * (qc + 1)],
                                g_sb[:, 0:1],
                                rtb[jq][:, 512 * qc:512 * (qc + 1)],
                                MULT, ADD)
                    return
                else:
                    zi = main.tile([1, S], f32, tag="zi", bufs=2,
                                   name=f"zi{h}")
                    zbc = main.tile([64, S], f32, tag="zb", bufs=2,
                                    name=f"zb{h}")
                    nc.vector.reciprocal_approx_fast(zi[0:1, :], o_ps[0:1, :])
                    nc.gpsimd.partition_broadcast(zbc[0:64, :], zi[0:1, :])
                    nc.vector.scalar_tensor_tensor(
                        rtb[jq][row:row + 64, :], o_ps[64:128, :], 1.0,
                        zbc[0:64, :], BYPASS, MULT)
                if h % 2 == 1:  # pair complete: s_in^T = rtb + gate*ctokT
                    nc.vector.scalar_tensor_tensor(
                        rtb[jq][:], ctok_c[jq][:], g_sb[:, 0:1], rtb[jq][:],
                        MULT, ADD)

            # flat emission; o(h) trails into head h+1 per the o-bank cycle
            for h in range(NH):
                for kt in range(8):
                    emit_sc(h, kt)
                    if h > 0:
                        if kt == 0:
                            emit_o(h - 1, 2)
                        elif kt == 1:
                            emit_o(h - 1, 3)
                        elif kt == 2:
                            emit_norm(h - 1)
                    if (h, kt) in ilv:
                        ilv[(h, kt)]()
                    if kt == 6:
                        emit_o(h, 0)
                    elif kt == 7:
                        emit_o(h, 1)
            # ---- tail (same pool: proj/conv accumulators ride the
            # sc/o tags as [128,512] halves; proj j0-2 partials fill the
            # end of the exp stream before the last o matmuls) ----
            rtb_v = [rtb[j][:].rearrange("p (g i two) -> p g two i",
                                         g=4, two=2) for j in range(4)]
            s2d_sb = [[main.tile([128, 512], bf16, tag=f"s2d{par}{g}",
                                 name=f"s2d{par}{g}") for g in range(4)]
                      for par in range(2)]
            ostp = [main.tile([128, 4 * 512], bf16, tag=f"ostp{par}",
                              name=f"ostp{par}") for par in range(2)]
            pj = {}

            def emit_pre2(pairs):
                tl_ = ps.tile([128, 1024], f32, tag="sc", bufs=3,
                              name=f"pj{pairs[0][0]}{pairs[0][1]}")
                for i, (par, g) in enumerate(pairs):
                    acc = tl_[:, 512 * i:512 * (i + 1)]
                    pj[(par, g)] = acc
                    for j in range(3):
                        nc.tensor.matmul(acc, rtb_v[j][:, g, par, :],
                                         wp_c[j][:, :], start=(j == 0),
                                         stop=False)

            def emit_fin(par, g, eng):
                acc = pj.pop((par, g), None)
                if acc is None:
                    # not pre-started: full 4-chain into the o-tag tile half
                    if "otl" not in pj:
                        pj["otl"] = ps.tile([128, 1024], f32, tag="o",
                                            bufs=1, name="pjo")
                    acc = pj["otl"][:, 512 * (g % 2):512 * (g % 2 + 1)]
                    for j in range(4):
                        nc.tensor.matmul(acc, rtb_v[j][:, g, par, :],
                                         wp_c[j][:, :], start=(j == 0),
                                         stop=(j == 3))
                else:
                    nc.tensor.matmul(acc, rtb_v[3][:, g, par, :],
                                     wp_c[3][:, :], start=False, stop=True)
                eng(s2d_sb[par][g][:], acc)

            def emit_conv_s(par):
                for ocp in range(2):
                    tl_ = ps.tile([128, 1024], f32, tag="sc", bufs=3,
                                  name=f"cv{par}{ocp}")
                    for i in range(2):
                        oc = 2 * ocp + i
                        acc = tl_[:, 512 * i:512 * (i + 1)]
                        for g in range(4):
                            nc.tensor.matmul(
                                acc, wcs_c[g][:, 128 * oc:128 * (oc + 1)],
                                s2d_sb[par][g][:],
                                start=(g == 0), stop=(g == 3))
                        eng = (nc.vector.tensor_copy if oc % 2 == 0
                               else nc.scalar.copy)
                        eng(ostp[par][:, 512 * oc:512 * (oc + 1)], acc)
                    out3 = out_p.rearrange("(oc p) s -> p oc s", oc=4)
                    src3 = ostp[par][:].rearrange("p (oc s) -> p oc s", oc=4)
                    nc.sync.dma_start(
                        out3[:, 2 * ocp:2 * ocp + 2,
                             512 * par:512 * (par + 1)],
                        src3[:, 2 * ocp:2 * ocp + 2, :])

            emit_pre2([(0, 0), (0, 1)])
            emit_o(NH - 1, 2)
            emit_pre2([(0, 2), (0, 3)])
            emit_o(NH - 1, 3)
            emit_norm(NH - 1, split=True)
            emit_pre2([(1, 0), (1, 1)])
            # fins for s2d column half 0 (g0,g1) only need the qc0 half of
            # the last chain; g2,g3 the qc1 half
            for par, g in [(0, 0), (0, 1), (1, 0), (1, 1),
                           (0, 2), (0, 3), (1, 2), (1, 3)]:
                emit_fin(par, g, nc.vector.tensor_copy if g % 2 == 0
                         else nc.scalar.copy)
            emit_conv_s(0)
            emit_conv_s(1)

    nc.compile()
    _CACHE["nc"] = nc
    return nc


def _shard_inputs(content_feat, components, pos_emb, Wq, Wkv, Wproj, bproj,
                  Wconv, bconv):
    import ml_dtypes

    bf = ml_dtypes.bfloat16
    f = np.float32
    pos2 = np.asarray(pos_emb, dtype=f).reshape(S, C)
    wq16 = np.asarray(Wq, dtype=f).astype(bf)
    wkv16 = np.asarray(Wkv, dtype=f).astype(bf)
    wp16 = np.asarray(Wproj, dtype=f).astype(bf)
    wcT = np.ascontiguousarray(np.asarray(Wconv, dtype=f).T)
    wcs16 = np.ascontiguousarray(wcT[:C]).astype(bf)
    in_maps = []
    for core in range(N_CORES):
        b, n = core // 4, core % 4
        ctokT = np.ascontiguousarray(
            (np.asarray(content_feat[b], dtype=f).reshape(S, C) + pos2).T)
        compT = np.ascontiguousarray(
            (np.asarray(components[n, b], dtype=f).reshape(S, C) + pos2).T)
        in_maps.append({
            "comp16": compT.astype(bf),
            "ctok16": ctokT.astype(bf),
            "cf16": np.ascontiguousarray(
                np.asarray(content_feat[b], dtype=f).reshape(C, S)).astype(bf),
            "wq16": wq16,
            "wkv16": wkv16,
            "wp16": wp16,
            "wcs16": wcs16,
            "wcc16": np.ascontiguousarray(
                wcT[C:, 128 * n:128 * (n + 1)]).astype(bf),
            "gate": np.full((128, 1), 1.0 if n == 0 else 0.0, dtype=f),
        })
    return in_maps


def _run(trace=False, **inputs):
    from concourse.bass_utils import run_bass_kernel_spmd

    nc = _build()
    in_maps = _shard_inputs(**inputs)
    res = run_bass_kernel_spmd(nc, in_maps, list(range(N_CORES)), trace=trace)
    outs = [np.asarray(res.results[i]["out_p"], dtype=np.float64)
            for i in range(N_CORES)]
    out = np.stack([outs[0] + outs[1] + outs[2] + outs[3],
                    outs[4] + outs[5] + outs[6] + outs[7]], axis=0)
    for core in range(N_CORES):
        b, n = core // 4, core % 4
        out[b, 128 * n:128 * (n + 1), :] += np.asarray(
            res.results[core]["out_cf"], dtype=np.float64)
    # host-side affine constants: out += bconv[o] + ws[o]*bproj[i%512]
    # with ws[o] = sum_c Wconv[o, c<C]  (bproj enters via the conv s-half)
    Wconv = np.asarray(inputs["Wconv"], dtype=np.float64)
    bproj = np.asarray(inputs["bproj"], dtype=np.float64)
    bconv = np.asarray(inputs["bconv"], dtype=np.float64)
    ws = Wconv[:, :C].sum(axis=1)
    bias_img = bconv[:, None] + np.outer(ws, np.concatenate([bproj, bproj]))
    out += bias_img[None, :, :]
    return out.reshape(B, C, H, W).astype(np.float32), res


def kernel(**inputs):
    out, _ = _run(trace=False, **inputs)
    return out
